# revision 15
# baseline (speedup 1.0000x reference)
"""Fused multi-head attention block on 8 TRN2 NeuronCores.

reference: qkv = x@Wqkv+b; q,k rmsnorm'd per head; softmax(q k^T/sqrt(hd)) v; proj.
Shapes: x [2,2048,1024], H=16 heads, hd=64.

Distribution (no collectives): 8 cores = 2 batches x 4 head-groups (4 heads each).
Core c: batch b=c//4, heads 4g..4g+3 (g=c%4). Each core computes the partial
projection output (proj_w row-sharded over its heads) for its batch; the host
sums the 4 partials per batch and adds proj_b.

Per-core pipeline (bf16 matmul operands, f32 PSUM accumulation):
  A) x (bf16, host-converted) -> PE-transpose -> xT ; qkv GEMM; rmsnorm(q,k)
     with rstd computed on DVE only (seed (3-m)/2 + 3 Newton steps - no ACT
     table switch vs the softmax exp); PE-transpose normed qk -> qT,kT bf16
     (norm weights folded into the evict); v (+bias) -> [ones|v] bf16 blocks.
  B) per (head, qtok-half 1024, ktile): S^T = kT^T qT (2 MMs into a 2-bank
     psum), one exp (scale=1/8, no max subtraction; scores in [-6,6]) -> P^T
     bf16, AV with the REUSED stationary [ones|v]: out rows 0:64 = denominator
     broadcast, rows 64:128 = unnormalized out^T, accumulated over ktiles.
     Epilogue: aT = out^T * recip(denom) (PSUM base-64 x SBUF base-0 is legal).
  C) partial projection straight from aT (K=64 per head), DMA out.
"""

from contextlib import ExitStack

import ml_dtypes
import numpy as np

import concourse.bass as bass
import concourse.mybir as mybir
import concourse.tile as tile
from concourse import bacc
from concourse.bass_utils import run_bass_kernel_spmd
from concourse.masks import make_identity

B, N, C = 2, 2048, 1024
H, HD = 16, 64
HPC = 4                 # heads per core
NT = N // 128           # 16 token tiles
KT8 = C // 128          # 8 contraction tiles for the qkv GEMM
QK = 2 * HPC * HD       # 512 qk channels per core
V = HPC * HD            # 256 v channels per core
EPS = 1e-6
F32 = mybir.dt.float32
BF16 = mybir.dt.bfloat16
AF = mybir.ActivationFunctionType
MUL = mybir.AluOpType.mult
ADD = mybir.AluOpType.add


def build_nc(dbg=False):
    nc = bacc.Bacc("TRN2", target_bir_lowering=False, debug=False)

    x_ext = nc.declare_dram_parameter("x", [N, C], BF16, isOutput=False)
    wqkv_ext = nc.declare_dram_parameter("wqkv", [C, QK + V], BF16, isOutput=False)
    bqkv_ext = nc.declare_dram_parameter("bqkv", [QK + V], F32, isOutput=False)
    normw_ext = nc.declare_dram_parameter("normw", [QK], F32, isOutput=False)
    wproj_ext = nc.declare_dram_parameter("wproj", [HD, HPC, C], BF16, isOutput=False)
    out_ext = nc.declare_dram_parameter("out", [N, C], F32, isOutput=True)

    with tile.TileContext(nc) as tc, ExitStack() as ctx:
        singles = ctx.enter_context(tc.tile_pool(name="singles", bufs=1))

        ident = singles.tile([128, 128], BF16, tag="ident")
        make_identity(nc, ident)

        wqkv_sb = singles.tile([128, KT8, QK + V], BF16, tag="wqkv")
        for kt in range(KT8):
            nc.sync.dma_start(
                out=wqkv_sb[:, kt, :], in_=wqkv_ext[kt * 128:(kt + 1) * 128, :]
            )
        # proj weights live on partitions 64:128 to match the AV output rows
        wproj_sb = singles.tile([128, HPC, C], BF16, tag="wproj")
        nc.sync.dma_start(out=wproj_sb[64:128, :, :], in_=wproj_ext[:, :, :])
        bias_sb = singles.tile([128, QK + V], F32, tag="bias")
        nc.sync.dma_start(out=bias_sb, in_=bqkv_ext[:].partition_broadcast(128))
        # normw_sb[p, cb] = normw[cb*128 + p]
        normw_sb = singles.tile([128, 4], F32, tag="normw")
        nc.sync.dma_start(out=normw_sb, in_=normw_ext[:].rearrange("(b p) -> p b", p=128))

        # persistent activations
        qT = singles.tile([128, 2, N], BF16, tag="qT")     # channel-major q
        kT = singles.tile([128, 2, N], BF16, tag="kT")     # channel-major k
        # AV stationary blocks: [:, t, h, 0:64] = 1.0, [:, t, h, 64:128] = v_h
        vaug = singles.tile([128, NT, HPC, 128], BF16, tag="vaug")
        nc.vector.memset(vaug, 1.0)
        # normalized attention output, transposed: rows 64:128 of block h hold
        # head h's channels; columns are tokens
        aT = singles.tile([128, HPC, N], BF16, tag="aT")

        # ---------------- phase A: qkv + rmsnorm + transposes ----------------
        with ExitStack() as actx:
            xpool = actx.enter_context(tc.tile_pool(name="xin", bufs=3))
            xtpool = actx.enter_context(tc.tile_pool(name="xt", bufs=2))
            qkpool = actx.enter_context(tc.tile_pool(name="qksb", bufs=3))
            stpool = actx.enter_context(tc.tile_pool(name="stats", bufs=8))
            ptr = actx.enter_context(tc.tile_pool(name="ptr", bufs=2, space="PSUM"))
            pqk = actx.enter_context(tc.tile_pool(name="pqk", bufs=2, space="PSUM"))
            pv = actx.enter_context(tc.tile_pool(name="pv", bufs=2, space="PSUM"))

            for t in range(NT):
                ts = slice(t * 128, (t + 1) * 128)
                xin = xpool.tile([128, C], BF16, tag="xin")
                nc.sync.dma_start(out=xin, in_=x_ext[ts, :])

                # transpose x tile -> xT blocks [C-part, tok]
                xt = xtpool.tile([128, KT8, 128], BF16, tag="xt")
                for kt in range(KT8):
                    p_tr = ptr.tile([128, 128], BF16, tag="ptr")
                    nc.tensor.transpose(p_tr, xin[:, kt * 128:(kt + 1) * 128], ident)
                    if kt % 2 == 0:
                        nc.vector.tensor_copy(xt[:, kt, :], p_tr)
                    else:
                        nc.scalar.copy(xt[:, kt, :], p_tr)

                # qkv GEMM for this token tile
                p_qk = pqk.tile([128, QK], F32, tag="pqk")
                p_v = pv.tile([128, V], F32, tag="pv")
                for kt in range(KT8):
                    nc.tensor.matmul(
                        p_qk, xt[:, kt, :], wqkv_sb[:, kt, 0:QK],
                        start=(kt == 0), stop=(kt == KT8 - 1),
                    )
                for kt in range(KT8):
                    nc.tensor.matmul(
                        p_v, xt[:, kt, :], wqkv_sb[:, kt, QK:QK + V],
                        start=(kt == 0), stop=(kt == KT8 - 1),
                    )

                # eviction with bias add
                qk_sb = qkpool.tile([128, QK], F32, tag="qksb")
                nc.vector.tensor_add(qk_sb, p_qk, bias_sb[:, 0:QK])
                nc.vector.tensor_add(
                    vaug[:, t, :, 64:128],
                    p_v.rearrange("p (h d) -> p h d", d=HD),
                    bias_sb[:, QK:QK + V].rearrange("p (h d) -> p h d", d=HD),
                )

                # rmsnorm over each 64-channel head group of q and k.
                # rstd = 1/sqrt(m), m = ssq/64 + eps, all on DVE: seed (3-m)/2
                # then 3 Newton steps y *= 1.5 - 0.5*m*y^2  (m stays in ~[0.3,3])
                sq = qkpool.tile([128, QK], F32, tag="sq")
                nc.vector.tensor_mul(sq, qk_sb, qk_sb)
                m = stpool.tile([128, 2 * HPC], F32, tag="m")
                nc.vector.tensor_reduce(
                    m, sq.rearrange("p (g d) -> p g d", d=HD),
                    axis=mybir.AxisListType.X, op=mybir.AluOpType.add,
                )
                nc.vector.tensor_scalar(m, m, 1.0 / HD, EPS, op0=MUL, op1=ADD)
                y = stpool.tile([128, 2 * HPC], F32, tag="y")
                nc.vector.tensor_scalar(y, m, -0.5, 1.5, op0=MUL, op1=ADD)
                t2 = stpool.tile([128, 2 * HPC], F32, tag="t2")
                for _ in range(3):
                    nc.vector.tensor_mul(t2, y, y)
                    nc.vector.tensor_mul(t2, t2, m)
                    nc.vector.tensor_scalar(t2, t2, -0.5, 1.5, op0=MUL, op1=ADD)
                    nc.vector.tensor_mul(y, y, t2)

                qkb = qkpool.tile([128, QK], BF16, tag="qkb")
                for g in range(2 * HPC):
                    nc.vector.tensor_scalar_mul(
                        qkb[:, g * HD:(g + 1) * HD],
                        qk_sb[:, g * HD:(g + 1) * HD],
                        y[:, g:g + 1],
                    )

                # transpose normed qk -> qT/kT, folding in the norm weights
                for cb in range(4):
                    p_tr = ptr.tile([128, 128], BF16, tag="ptr")
                    nc.tensor.transpose(p_tr, qkb[:, cb * 128:(cb + 1) * 128], ident)
                    dst = qT[:, cb, ts] if cb < 2 else kT[:, cb - 2, ts]
                    nc.vector.tensor_scalar_mul(dst, p_tr, normw_sb[:, cb:cb + 1])

        # ---------------- phase B: attention ----------------
        with ExitStack() as bctx:
            spool = bctx.enter_context(tc.tile_pool(name="ps", bufs=2, space="PSUM"))
            opool = bctx.enter_context(tc.tile_pool(name="po", bufs=2, space="PSUM"))
            ptpool = bctx.enter_context(tc.tile_pool(name="pt", bufs=3))
            rpool = bctx.enter_context(tc.tile_pool(name="rec", bufs=2))

            for h in range(HPC):
                pb = (h % 2) * 64          # partition base of this head's channels
                cb = h // 2                # column block in qT/kT
                psl = slice(pb, pb + 64)
                for qh in range(2):        # 1024-token halves of the query axis
                    po = opool.tile([128, 2, 512], F32, tag="po")
                    for kt in range(NT):
                        ps = spool.tile([128, 1024], F32, tag="ps")
                        for i in range(2):
                            qsl = slice(qh * 1024 + i * 512, qh * 1024 + (i + 1) * 512)
                            nc.tensor.matmul(
                                ps[:, i * 512:(i + 1) * 512],
                                kT[psl, cb, kt * 128:(kt + 1) * 128],
                                qT[psl, cb, qsl],
                                start=True, stop=True,
                            )
                        pt = ptpool.tile([128, 1024], BF16, tag="pt")
                        nc.scalar.activation(pt, ps, AF.Exp, scale=0.125)
                        for i in range(2):
                            nc.tensor.matmul(
                                po[:, i, :],
                                vaug[:, kt, h, :],
                                pt[:, i * 512:(i + 1) * 512],
                                start=(kt == 0), stop=(kt == NT - 1),
                            )
                    # rows 0:64 = denominator (broadcast), 64:128 = out^T
                    rec = rpool.tile([128, 2, 512], F32, tag="rec")
                    nc.vector.reciprocal(rec[0:64, :, :], po[0:64, :, :])
                    nc.vector.tensor_mul(
                        aT[64:128, h, qh * 1024:(qh + 1) * 1024]
                          .rearrange("p (i q) -> p i q", i=2),
                        po[64:128, :, :],
                        rec[0:64, :, :],
                    )

        # ---------------- phase C: partial projection ----------------
        with ExitStack() as cctx:
            ppool = cctx.enter_context(tc.tile_pool(name="pp", bufs=3, space="PSUM"))
            outpool = cctx.enter_context(tc.tile_pool(name="outsb", bufs=4))

            for t in range(NT):
                ts = slice(t * 128, (t + 1) * 128)
                for jg in range(2):
                    pp = ppool.tile([128, 512], F32, tag="pp")
                    for h in range(HPC):
                        nc.tensor.matmul(
                            pp, aT[64:128, h, ts],
                            wproj_sb[64:128, h, jg * 512:(jg + 1) * 512],
                            start=(h == 0), stop=(h == HPC - 1),
                        )
                    outsb = outpool.tile([128, 512], F32, tag="outsb")
                    nc.vector.tensor_copy(outsb, pp)
                    nc.sync.dma_start(
                        out=out_ext[ts, jg * 512:(jg + 1) * 512], in_=outsb
                    )

    nc.finalize()
    return nc


def make_in_maps(x, qkv_w, qkv_b, q_norm_w, k_norm_w, proj_w, proj_b):
    """Shard the full inputs into the 8 per-core input maps."""
    bf = ml_dtypes.bfloat16
    in_maps = []
    for c in range(8):
        b, g = c // 4, c % 4
        ch = np.arange(4 * g * HD, 4 * (g + 1) * HD)          # this core's head channels
        wqkv_c = np.concatenate(
            [qkv_w[:, ch], qkv_w[:, C + ch], qkv_w[:, 2 * C + ch]], axis=1
        )
        bqkv_c = np.concatenate([qkv_b[ch], qkv_b[C + ch], qkv_b[2 * C + ch]])
        normw = np.concatenate([np.tile(q_norm_w, HPC), np.tile(k_norm_w, HPC)])
        # wproj rows for this core, as [d 64, head 4, C]
        wproj_c = proj_w[ch, :].reshape(HPC, HD, C).transpose(1, 0, 2)
        in_maps.append({
            "x": np.ascontiguousarray(x[b]).astype(bf),
            "wqkv": np.ascontiguousarray(wqkv_c).astype(bf),
            "bqkv": np.ascontiguousarray(bqkv_c, np.float32),
            "normw": np.ascontiguousarray(normw, np.float32),
            "wproj": np.ascontiguousarray(wproj_c).astype(bf),
        })
    return in_maps


_NC_CACHE = []


def kernel(x, qkv_w, qkv_b, q_norm_w, k_norm_w, proj_w, proj_b,
           _run_kwargs=None, _res_box=None):
    x = np.asarray(x); qkv_w = np.asarray(qkv_w); qkv_b = np.asarray(qkv_b)
    q_norm_w = np.asarray(q_norm_w); k_norm_w = np.asarray(k_norm_w)
    proj_w = np.asarray(proj_w); proj_b = np.asarray(proj_b)

    if not _NC_CACHE:
        _NC_CACHE.append(build_nc())
    nc = _NC_CACHE[0]
    in_maps = make_in_maps(x, qkv_w, qkv_b, q_norm_w, k_norm_w, proj_w, proj_b)
    res = run_bass_kernel_spmd(nc, in_maps, core_ids=list(range(8)),
                               **(_run_kwargs or {}))
    if _res_box is not None:
        _res_box["res"] = res
    out = np.zeros((B, N, C), np.float32)
    for c in range(8):
        out[c // 4] += res.results[c]["out"]
    out += proj_b[None, None, :].astype(np.float32)
    return out


if __name__ == "__main__":
    rng = np.random.default_rng(0)
    x = rng.standard_normal((B, N, C)).astype(np.float32)
    qkv_w = (rng.standard_normal((C, 3 * C)) / np.sqrt(C)).astype(np.float32)
    qkv_b = np.zeros((3 * C,), np.float32)
    qn = np.ones((HD,), np.float32)
    kn = np.ones((HD,), np.float32)
    proj_w = (rng.standard_normal((C, C)) / np.sqrt(C)).astype(np.float32)
    proj_b = np.zeros((C,), np.float32)
    out = kernel(x, qkv_w, qkv_b, qn, kn, proj_w, proj_b)
    print("out", out.shape, out.dtype, float(np.abs(out).mean()))


# revision 20
# speedup vs baseline: 1.1253x; 1.1253x over previous
"""Fused multi-head attention block on 8 TRN2 NeuronCores.

reference: qkv = x@Wqkv+b; q,k rmsnorm'd per head; softmax(q k^T/sqrt(hd)) v; proj.
Shapes: x [2,2048,1024], H=16 heads, hd=64.

Distribution (no collectives): 8 cores = 2 batches x 4 head-groups (4 heads each).
Core c: batch b=c//4, heads 4g..4g+3 (g=c%4). Each core computes the partial
projection output (proj_w row-sharded over its heads) for its batch; the host
sums the 4 partials per batch and adds proj_b.

Per-core pipeline (bf16 matmul operands, f32 PSUM accumulation):
  A) x (bf16, host-converted) -> PE-transpose (4 packed per PSUM bank) -> xT;
     qkv GEMM; rmsnorm(q,k) with rstd on DVE only (seed (3-m)/2 + Newton);
     one stride-0-broadcast multiply applies rstd; PE-transpose normed qk ->
     qT,kT bf16 (norm weights folded into the evict); v (+bias) -> vaug.
  B) per (head, qtok-half 1024): software-pipelined over ktiles: scores
     S^T(kt) = kT^T qT (2 MMs), exp(kt) on ACT (scale=1/8, no max subtract,
     scores in [-6,6]), AV(kt-1) accumulates with the REUSED stationary
     vaug = [v|ones] (even heads) / [ones|v] (odd heads): out rows give the
     unnormalized out^T on one 64-partition half and the broadcast softmax
     denominator on the other. Epilogue: aT = out^T * approx-recip(denom).
     Head parity places outputs so aT packs head pairs on full 128 partitions.
  C) partial projection straight from aT (2 MMs, K=128), DMA out.
"""

from contextlib import ExitStack

import ml_dtypes
import numpy as np

import concourse.bass as bass
import concourse.mybir as mybir
import concourse.tile as tile
from concourse import bacc
from concourse.bass_utils import run_bass_kernel_spmd
from concourse.masks import make_identity

B, N, C = 2, 2048, 1024
H, HD = 16, 64
HPC = 4                 # heads per core
NT = N // 128           # 16 token tiles
KT8 = C // 128          # 8 contraction tiles for the qkv GEMM
QK = 2 * HPC * HD       # 512 qk channels per core
V = HPC * HD            # 256 v channels per core
EPS = 1e-6
F32 = mybir.dt.float32
BF16 = mybir.dt.bfloat16
AF = mybir.ActivationFunctionType
MUL = mybir.AluOpType.mult
ADD = mybir.AluOpType.add


def bcast_inner(ap, n):
    """Append a stride-0 inner dim of size n to a 2D AP."""
    return bass.AP(tensor=ap.tensor, offset=ap.offset,
                   ap=[list(ap.ap[0]), list(ap.ap[1]), [0, n]])


def build_nc(dbg=False):
    nc = bacc.Bacc("TRN2", target_bir_lowering=False, debug=False)

    x_ext = nc.declare_dram_parameter("x", [N, C], BF16, isOutput=False)
    wqkv_ext = nc.declare_dram_parameter("wqkv", [C, QK + V], BF16, isOutput=False)
    bqkv_ext = nc.declare_dram_parameter("bqkv", [QK + V], F32, isOutput=False)
    normw_ext = nc.declare_dram_parameter("normw", [QK], F32, isOutput=False)
    wproj_ext = nc.declare_dram_parameter("wproj", [V // 2, 2, C], BF16, isOutput=False)
    out_ext = nc.declare_dram_parameter("out", [N, C], F32, isOutput=True)
    if dbg:
        dbg_q = nc.declare_dram_parameter("dbg_q", [128, 2, N], F32, isOutput=True)
        dbg_k = nc.declare_dram_parameter("dbg_k", [128, 2, N], F32, isOutput=True)
        dbg_v = nc.declare_dram_parameter("dbg_v", [128, NT, HPC, 128], F32, isOutput=True)
        dbg_a = nc.declare_dram_parameter("dbg_a", [128, 2, N], F32, isOutput=True)

    with tile.TileContext(nc) as tc, ExitStack() as ctx:
        singles = ctx.enter_context(tc.tile_pool(name="singles", bufs=1))

        ident = singles.tile([128, 128], BF16, tag="ident")
        make_identity(nc, ident)

        wqkv_sb = singles.tile([128, KT8, QK + V], BF16, tag="wqkv")
        for kt in range(KT8):
            nc.sync.dma_start(
                out=wqkv_sb[:, kt, :], in_=wqkv_ext[kt * 128:(kt + 1) * 128, :]
            )
        wproj_sb = singles.tile([128, 2, C], BF16, tag="wproj")
        nc.sync.dma_start(out=wproj_sb, in_=wproj_ext[:, :, :])
        bias_sb = singles.tile([128, QK + V], F32, tag="bias")
        nc.sync.dma_start(out=bias_sb, in_=bqkv_ext[:].partition_broadcast(128))
        # normw_sb[p, cb] = normw[cb*128 + p]
        normw_sb = singles.tile([128, 4], F32, tag="normw")
        nc.sync.dma_start(out=normw_sb, in_=normw_ext[:].rearrange("(b p) -> p b", p=128))

        # persistent activations
        qT = singles.tile([128, 2, N], BF16, tag="qT")     # channel-major q
        kT = singles.tile([128, 2, N], BF16, tag="kT")     # channel-major k
        # AV stationary blocks: even head s: [v_s | ones]; odd: [ones | v_s]
        vaug = singles.tile([128, NT, HPC, 128], BF16, tag="vaug")
        nc.vector.memset(vaug, 1.0)
        # normalized attention output, transposed; block hb packs heads 2hb,2hb+1
        aT = singles.tile([128, 2, N], BF16, tag="aT")

        # ---------------- phase A: qkv + rmsnorm + transposes ----------------
        with ExitStack() as actx:
            xpool = actx.enter_context(tc.tile_pool(name="xin", bufs=3))
            xtpool = actx.enter_context(tc.tile_pool(name="xt", bufs=2))
            qkpool = actx.enter_context(tc.tile_pool(name="qksb", bufs=3))
            stpool = actx.enter_context(tc.tile_pool(name="stats", bufs=6))
            ptr = actx.enter_context(tc.tile_pool(name="ptr", bufs=3, space="PSUM"))
            pqk = actx.enter_context(tc.tile_pool(name="pqk", bufs=2, space="PSUM"))
            pv = actx.enter_context(tc.tile_pool(name="pv", bufs=2, space="PSUM"))

            for t in range(NT):
                ts = slice(t * 128, (t + 1) * 128)
                xin = xpool.tile([128, C], BF16, tag="xin")
                nc.sync.dma_start(out=xin, in_=x_ext[ts, :])

                # transpose x tile -> xT blocks [C-part, tok]; 4 per PSUM bank,
                # evicted with one ScalarE copy each
                xt = xtpool.tile([128, KT8, 128], BF16, tag="xt")
                for half in range(2):
                    p_tr = ptr.tile([128, 4, 128], BF16, tag="ptr")
                    for j in range(4):
                        kt = half * 4 + j
                        nc.tensor.matmul(
                            p_tr[:, j, :], xin[:, kt * 128:(kt + 1) * 128], ident,
                            is_transpose=True, start=(j == 0), stop=(j == 3),
                        )
                    nc.scalar.copy(xt[:, half * 4:half * 4 + 4, :], p_tr)

                # qkv GEMM for this token tile
                p_qk = pqk.tile([128, QK], F32, tag="pqk")
                p_v = pv.tile([128, V], F32, tag="pv")
                for kt in range(KT8):
                    nc.tensor.matmul(
                        p_qk, xt[:, kt, :], wqkv_sb[:, kt, 0:QK],
                        start=(kt == 0), stop=(kt == KT8 - 1),
                    )
                for kt in range(KT8):
                    nc.tensor.matmul(
                        p_v, xt[:, kt, :], wqkv_sb[:, kt, QK:QK + V],
                        start=(kt == 0), stop=(kt == KT8 - 1),
                    )

                # evictions with bias add; v goes to its parity-dependent half
                qk_sb = qkpool.tile([128, QK], F32, tag="qksb")
                nc.vector.tensor_add(qk_sb, p_qk, bias_sb[:, 0:QK])
                pv3 = p_v.rearrange("p (h d) -> p h d", d=HD)
                bv3 = bias_sb[:, QK:QK + V].rearrange("p (h d) -> p h d", d=HD)
                nc.vector.tensor_add(vaug[:, t, 0::2, 0:HD], pv3[:, 0::2, :],
                                     bv3[:, 0::2, :])
                nc.vector.tensor_add(vaug[:, t, 1::2, HD:128], pv3[:, 1::2, :],
                                     bv3[:, 1::2, :])

                # rmsnorm: rstd = 1/sqrt(m), m = ssq/64 + eps, DVE only:
                # seed (3-m)/2 then 2 Newton steps y *= 1.5 - 0.5*m*y^2
                sq = qkpool.tile([128, QK], F32, tag="sq")
                nc.vector.tensor_mul(sq, qk_sb, qk_sb)
                m = stpool.tile([128, 2 * HPC], F32, tag="m")
                nc.vector.tensor_reduce(
                    m, sq.rearrange("p (g d) -> p g d", d=HD),
                    axis=mybir.AxisListType.X, op=mybir.AluOpType.add,
                )
                nc.vector.tensor_scalar(m, m, 1.0 / HD, EPS, op0=MUL, op1=ADD)
                y = stpool.tile([128, 2 * HPC], F32, tag="y")
                nc.vector.tensor_scalar(y, m, -0.5, 1.5, op0=MUL, op1=ADD)
                t2 = stpool.tile([128, 2 * HPC], F32, tag="t2")
                for _ in range(2):
                    nc.vector.tensor_mul(t2, y, y)
                    nc.vector.tensor_mul(t2, t2, m)
                    nc.vector.tensor_scalar(t2, t2, -0.5, 1.5, op0=MUL, op1=ADD)
                    nc.vector.tensor_mul(y, y, t2)

                # apply rstd with one stride-0-broadcast multiply (f32 -> bf16)
                qkb = qkpool.tile([128, QK], BF16, tag="qkb")
                nc.vector.tensor_tensor(
                    qkb.rearrange("p (g d) -> p g d", d=HD),
                    qk_sb.rearrange("p (g d) -> p g d", d=HD),
                    bcast_inner(y, HD), op=MUL,
                )

                # transpose normed qk -> qT/kT, folding in the norm weights
                for half in range(2):
                    p_tr = ptr.tile([128, 4, 128], BF16, tag="ptr")
                    for j in range(2):
                        cb = half * 2 + j
                        nc.tensor.matmul(
                            p_tr[:, j, :], qkb[:, cb * 128:(cb + 1) * 128], ident,
                            is_transpose=True, start=(j == 0), stop=(j == 1),
                        )
                    for j in range(2):
                        cb = half * 2 + j
                        dst = qT[:, cb, ts] if cb < 2 else kT[:, cb - 2, ts]
                        nc.vector.tensor_scalar_mul(dst, p_tr[:, j, :],
                                                    normw_sb[:, cb:cb + 1])

        # ---------------- phase B: attention ----------------
        with ExitStack() as bctx:
            spool = bctx.enter_context(tc.tile_pool(name="ps", bufs=2, space="PSUM"))
            opool = bctx.enter_context(tc.tile_pool(name="po", bufs=2, space="PSUM"))
            ptpool = bctx.enter_context(tc.tile_pool(name="pt", bufs=3))
            rpool = bctx.enter_context(tc.tile_pool(name="rec", bufs=2))

            for h in range(HPC):
                pb = (h % 2) * 64          # partition base of this head's channels
                cb = h // 2                # column block in qT/kT
                psl = slice(pb, pb + 64)
                # even head: o_un^T on rows 0:64, denom on 64:128; odd: swapped
                osl = slice(0, 64) if h % 2 == 0 else slice(64, 128)
                dsl = slice(64, 128) if h % 2 == 0 else slice(0, 64)
                for qh in range(2):        # 1024-token halves of the query axis
                    po = opool.tile([128, 2, 512], F32, tag="po")
                    prev = None
                    for kt in range(NT):
                        ps = spool.tile([128, 1024], F32, tag="ps")
                        for i in range(2):
                            qsl = slice(qh * 1024 + i * 512, qh * 1024 + (i + 1) * 512)
                            nc.tensor.matmul(
                                ps[:, i * 512:(i + 1) * 512],
                                kT[psl, cb, kt * 128:(kt + 1) * 128],
                                qT[psl, cb, qsl],
                                start=True, stop=True,
                            )
                        pt = ptpool.tile([128, 1024], BF16, tag="pt")
                        nc.scalar.activation(pt, ps, AF.Exp, scale=0.125)
                        if prev is not None:
                            ppt, pkt = prev
                            for i in range(2):
                                nc.tensor.matmul(
                                    po[:, i, :], vaug[:, pkt, h, :],
                                    ppt[:, i * 512:(i + 1) * 512],
                                    start=(pkt == 0), stop=False,
                                )
                        prev = (pt, kt)
                    ppt, pkt = prev
                    for i in range(2):
                        nc.tensor.matmul(
                            po[:, i, :], vaug[:, pkt, h, :],
                            ppt[:, i * 512:(i + 1) * 512],
                            start=False, stop=True,
                        )
                    # normalize: aT rows osl = po rows osl * recip(po rows dsl)
                    rec = rpool.tile([128, 2, 512], F32, tag="rec")
                    nc.vector.reciprocal_approx_fast(rec, po)
                    nc.vector.tensor_mul(
                        aT[osl, cb, qh * 1024:(qh + 1) * 1024]
                          .rearrange("p (i q) -> p i q", i=2),
                        po[osl, :, :],
                        rec[dsl, :, :],
                    )

        # ---------------- phase C: partial projection ----------------
        with ExitStack() as cctx:
            ppool = cctx.enter_context(tc.tile_pool(name="pp", bufs=3, space="PSUM"))
            outpool = cctx.enter_context(tc.tile_pool(name="outsb", bufs=4))

            for t in range(NT):
                ts = slice(t * 128, (t + 1) * 128)
                for jg in range(2):
                    pp = ppool.tile([128, 512], F32, tag="pp")
                    for hb in range(2):
                        nc.tensor.matmul(
                            pp, aT[:, hb, ts],
                            wproj_sb[:, hb, jg * 512:(jg + 1) * 512],
                            start=(hb == 0), stop=(hb == 1),
                        )
                    outsb = outpool.tile([128, 512], F32, tag="outsb")
                    nc.vector.tensor_copy(outsb, pp)
                    nc.sync.dma_start(
                        out=out_ext[ts, jg * 512:(jg + 1) * 512], in_=outsb
                    )

        if dbg:
            with tc.tile_pool(name="dbgp", bufs=1) as dp:
                for nm, dst, src in (("q", dbg_q, qT), ("k", dbg_k, kT),
                                     ("a", dbg_a, aT)):
                    f = dp.tile([128, 2, N], F32, tag=f"f{nm}", name=f"f{nm}")
                    nc.vector.tensor_copy(f, src)
                    nc.sync.dma_start(out=dst[:, :, :], in_=f)
                fv = dp.tile([128, NT, HPC, 128], F32, tag="fv", name="fv")
                nc.vector.tensor_copy(fv, vaug)
                nc.sync.dma_start(out=dbg_v[:, :, :, :], in_=fv)

    nc.finalize()
    return nc


def make_in_maps(x, qkv_w, qkv_b, q_norm_w, k_norm_w, proj_w, proj_b):
    """Shard the full inputs into the 8 per-core input maps."""
    bf = ml_dtypes.bfloat16
    in_maps = []
    for c in range(8):
        b, g = c // 4, c % 4
        ch = np.arange(4 * g * HD, 4 * (g + 1) * HD)          # this core's head channels
        wqkv_c = np.concatenate(
            [qkv_w[:, ch], qkv_w[:, C + ch], qkv_w[:, 2 * C + ch]], axis=1
        )
        bqkv_c = np.concatenate([qkv_b[ch], qkv_b[C + ch], qkv_b[2 * C + ch]])
        normw = np.concatenate([np.tile(q_norm_w, HPC), np.tile(k_norm_w, HPC)])
        # wproj rows for this core as [128 rows of head-pair, pair, C]
        wproj_c = proj_w[ch, :].reshape(2, V // 2, C).transpose(1, 0, 2)
        in_maps.append({
            "x": np.ascontiguousarray(x[b]).astype(bf),
            "wqkv": np.ascontiguousarray(wqkv_c).astype(bf),
            "bqkv": np.ascontiguousarray(bqkv_c, np.float32),
            "normw": np.ascontiguousarray(normw, np.float32),
            "wproj": np.ascontiguousarray(wproj_c).astype(bf),
        })
    return in_maps


_NC_CACHE = []


def kernel(x, qkv_w, qkv_b, q_norm_w, k_norm_w, proj_w, proj_b,
           _run_kwargs=None, _res_box=None):
    x = np.asarray(x); qkv_w = np.asarray(qkv_w); qkv_b = np.asarray(qkv_b)
    q_norm_w = np.asarray(q_norm_w); k_norm_w = np.asarray(k_norm_w)
    proj_w = np.asarray(proj_w); proj_b = np.asarray(proj_b)

    if not _NC_CACHE:
        _NC_CACHE.append(build_nc())
    nc = _NC_CACHE[0]
    in_maps = make_in_maps(x, qkv_w, qkv_b, q_norm_w, k_norm_w, proj_w, proj_b)
    res = run_bass_kernel_spmd(nc, in_maps, core_ids=list(range(8)),
                               **(_run_kwargs or {}))
    if _res_box is not None:
        _res_box["res"] = res
    out = np.zeros((B, N, C), np.float32)
    for c in range(8):
        out[c // 4] += res.results[c]["out"]
    out += proj_b[None, None, :].astype(np.float32)
    return out


if __name__ == "__main__":
    rng = np.random.default_rng(0)
    x = rng.standard_normal((B, N, C)).astype(np.float32)
    qkv_w = (rng.standard_normal((C, 3 * C)) / np.sqrt(C)).astype(np.float32)
    qkv_b = np.zeros((3 * C,), np.float32)
    qn = np.ones((HD,), np.float32)
    kn = np.ones((HD,), np.float32)
    proj_w = (rng.standard_normal((C, C)) / np.sqrt(C)).astype(np.float32)
    proj_b = np.zeros((C,), np.float32)
    out = kernel(x, qkv_w, qkv_b, qn, kn, proj_w, proj_b)
    print("out", out.shape, out.dtype, float(np.abs(out).mean()))


# revision 27
# speedup vs baseline: 1.3777x; 1.2242x over previous
"""Fused multi-head attention block on 8 TRN2 NeuronCores.

reference: qkv = x@Wqkv+b; q,k rmsnorm'd per head; softmax(q k^T/sqrt(hd)) v; proj.
Shapes: x [2,2048,1024], H=16 heads, hd=64.

Distribution (no collectives): 8 cores = 2 batches x 4 head-groups (4 heads each).
Core c: batch b=c//4, heads 4g..4g+3 (g=c%4). Each core computes the partial
projection output (proj_w row-sharded over its heads) for its batch; the host
sums the 4 partials per batch and adds proj_b.

Per-core pipeline (bf16 matmul operands, f32 PSUM accumulation):
  A) x (bf16, host-converted) -> PE-transpose (4 packed per PSUM bank) -> xT;
     qkv GEMM; rmsnorm(q,k) with rstd on DVE only (seed (3-m)/2 + Newton);
     one stride-0-broadcast multiply applies rstd; PE-transpose normed qk ->
     qT,kT bf16 (norm weights folded into the evict); v (+bias) -> vaug.
  B) per (head, qtok-half 1024): software-pipelined over ktiles: scores
     S^T(kt) = kT^T qT (2 MMs), exp(kt) on ACT (scale=1/8, no max subtract,
     scores in [-6,6]), AV(kt-1) accumulates with the REUSED stationary
     vaug = [v|ones] (even heads) / [ones|v] (odd heads): out rows give the
     unnormalized out^T on one 64-partition half and the broadcast softmax
     denominator on the other. Epilogue: aT = out^T * approx-recip(denom).
     Head parity places outputs so aT packs head pairs on full 128 partitions.
  C) partial projection straight from aT (2 MMs, K=128), DMA out.
"""

from contextlib import ExitStack

import ml_dtypes
import numpy as np

import concourse.bass as bass
import concourse.mybir as mybir
import concourse.tile as tile
from concourse import bacc
from concourse.bass_utils import run_bass_kernel_spmd
from concourse.masks import make_identity

B, N, C = 2, 2048, 1024
H, HD = 16, 64
HPC = 4                 # heads per core
NT = N // 128           # 16 token tiles
KT8 = C // 128          # 8 contraction tiles for the qkv GEMM
QK = 2 * HPC * HD       # 512 qk channels per core
V = HPC * HD            # 256 v channels per core
EPS = 1e-6
F32 = mybir.dt.float32
BF16 = mybir.dt.bfloat16
AF = mybir.ActivationFunctionType
MUL = mybir.AluOpType.mult
ADD = mybir.AluOpType.add


def bcast_inner(ap, n):
    """Append a stride-0 inner dim of size n to a 2D AP."""
    return bass.AP(tensor=ap.tensor, offset=ap.offset,
                   ap=[list(ap.ap[0]), list(ap.ap[1]), [0, n]])


def build_nc(dbg=False):
    nc = bacc.Bacc("TRN2", target_bir_lowering=False, debug=False)

    x_ext = nc.declare_dram_parameter("x", [N, C], BF16, isOutput=False)
    wqkv_ext = nc.declare_dram_parameter("wqkv", [C, QK + V], BF16, isOutput=False)
    bqkv_ext = nc.declare_dram_parameter("bqkv", [QK + V], F32, isOutput=False)
    normw_ext = nc.declare_dram_parameter("normw", [QK], F32, isOutput=False)
    wproj_ext = nc.declare_dram_parameter("wproj", [V // 2, 2, C], BF16, isOutput=False)
    out_ext = nc.declare_dram_parameter("out", [N, C], F32, isOutput=True)
    if dbg:
        dbg_q = nc.declare_dram_parameter("dbg_q", [128, 2, N], F32, isOutput=True)
        dbg_k = nc.declare_dram_parameter("dbg_k", [128, HPC, NT, 128], F32, isOutput=True)
        dbg_v = nc.declare_dram_parameter("dbg_v", [128, NT, HPC, 128], F32, isOutput=True)
        dbg_a = nc.declare_dram_parameter("dbg_a", [128, 2, N], F32, isOutput=True)

    with tile.TileContext(nc) as tc, ExitStack() as ctx:
        singles = ctx.enter_context(tc.tile_pool(name="singles", bufs=1))

        ident = singles.tile([128, 128], BF16, tag="ident")
        make_identity(nc, ident)

        wqkv_sb = singles.tile([128, KT8, QK + V], BF16, tag="wqkv")
        for kt in range(KT8):
            nc.sync.dma_start(
                out=wqkv_sb[:, kt, :], in_=wqkv_ext[kt * 128:(kt + 1) * 128, :]
            )
        wproj_sb = singles.tile([128, 2, C], BF16, tag="wproj")
        nc.sync.dma_start(out=wproj_sb, in_=wproj_ext[:, :, :])
        bias_sb = singles.tile([128, QK + V], F32, tag="bias")
        nc.sync.dma_start(out=bias_sb, in_=bqkv_ext[:].partition_broadcast(128))
        # normw_sb[p, cb] = normw[cb*128 + p]
        normw_sb = singles.tile([128, 4], F32, tag="normw")
        nc.sync.dma_start(out=normw_sb, in_=normw_ext[:].rearrange("(b p) -> p b", p=128))

        # persistent activations
        qT = singles.tile([128, 2, N], BF16, tag="qT")     # channel-major q
        # k^T stored per head, zero-padded to K=128 on the partition axis:
        # head h occupies rows (h%2)*64..+64 of [:, h, kt, :]; other rows stay
        # zero so the scores matmul runs with a full-K stationary (K=64
        # matmuls execute at half rate and keep the PE clock cold).
        zkT = singles.tile([128, HPC, NT, 128], BF16, tag="zkT")
        nc.vector.memset(zkT, 0.0)
        # AV stationary blocks: even head s: [v_s | ones]; odd: [ones | v_s]
        vaug = singles.tile([128, NT, HPC, 128], BF16, tag="vaug")
        nc.vector.memset(vaug, 1.0)
        # normalized attention output, transposed; block hb packs heads 2hb,2hb+1
        aT = singles.tile([128, 2, N], BF16, tag="aT")

        # ---------------- phase A: qkv + rmsnorm + transposes ----------------
        with ExitStack() as actx:
            xpool = actx.enter_context(tc.tile_pool(name="xin", bufs=3))
            xtpool = actx.enter_context(tc.tile_pool(name="xt", bufs=3))
            qkpool = actx.enter_context(tc.tile_pool(name="qksb", bufs=3))
            stpool = actx.enter_context(tc.tile_pool(name="stats", bufs=6))
            ptr = actx.enter_context(tc.tile_pool(name="ptr", bufs=3, space="PSUM"))
            pqk = actx.enter_context(tc.tile_pool(name="pqk", bufs=2, space="PSUM"))
            pv = actx.enter_context(tc.tile_pool(name="pv", bufs=2, space="PSUM"))

            for t in range(NT):
                ts = slice(t * 128, (t + 1) * 128)
                xin = xpool.tile([128, C], BF16, tag="xin")
                nc.sync.dma_start(out=xin, in_=x_ext[ts, :])

                # transpose x tile -> xT blocks [C-part, tok]; 4 per PSUM bank,
                # evicted with one ScalarE copy each
                xt = xtpool.tile([128, KT8, 128], BF16, tag="xt")
                for half in range(2):
                    p_tr = ptr.tile([128, 4, 128], BF16, tag="ptr")
                    for j in range(4):
                        kt = half * 4 + j
                        nc.tensor.matmul(
                            p_tr[:, j, :], xin[:, kt * 128:(kt + 1) * 128], ident,
                            is_transpose=True, start=(j == 0), stop=(j == 3),
                        )
                    nc.scalar.copy(xt[:, half * 4:half * 4 + 4, :], p_tr)

                # qkv GEMM for this token tile
                p_qk = pqk.tile([128, QK], F32, tag="pqk")
                p_v = pv.tile([128, V], F32, tag="pv")
                for kt in range(KT8):
                    nc.tensor.matmul(
                        p_qk, xt[:, kt, :], wqkv_sb[:, kt, 0:QK],
                        start=(kt == 0), stop=(kt == KT8 - 1),
                    )
                for kt in range(KT8):
                    nc.tensor.matmul(
                        p_v, xt[:, kt, :], wqkv_sb[:, kt, QK:QK + V],
                        start=(kt == 0), stop=(kt == KT8 - 1),
                    )

                # evictions with bias add; v goes to its parity-dependent half
                qk_sb = qkpool.tile([128, QK], F32, tag="qksb")
                nc.vector.tensor_add(qk_sb, p_qk, bias_sb[:, 0:QK])
                pv3 = p_v.rearrange("p (h d) -> p h d", d=HD)
                bv3 = bias_sb[:, QK:QK + V].rearrange("p (h d) -> p h d", d=HD)
                nc.vector.tensor_add(vaug[:, t, 0::2, 0:HD], pv3[:, 0::2, :],
                                     bv3[:, 0::2, :])
                nc.vector.tensor_add(vaug[:, t, 1::2, HD:128], pv3[:, 1::2, :],
                                     bv3[:, 1::2, :])

                # rmsnorm: rstd = 1/sqrt(m), m = ssq/64 + eps:
                # seed (3-m)/2 then 2 Newton steps y *= 1.5 - 0.5*m*y^2
                sq = qkpool.tile([128, QK], F32, tag="sq")
                nc.scalar.square(sq, qk_sb)
                m = stpool.tile([128, 2 * HPC], F32, tag="m")
                nc.vector.tensor_reduce(
                    m, sq.rearrange("p (g d) -> p g d", d=HD),
                    axis=mybir.AxisListType.X, op=mybir.AluOpType.add,
                )
                nc.vector.tensor_scalar(m, m, 1.0 / HD, EPS, op0=MUL, op1=ADD)
                y = stpool.tile([128, 2 * HPC], F32, tag="y")
                nc.vector.tensor_scalar(y, m, -0.5, 1.5, op0=MUL, op1=ADD)
                t2 = stpool.tile([128, 2 * HPC], F32, tag="t2")
                for _ in range(2):
                    nc.vector.tensor_mul(t2, y, y)
                    nc.vector.tensor_mul(t2, t2, m)
                    nc.vector.tensor_scalar(t2, t2, -0.5, 1.5, op0=MUL, op1=ADD)
                    nc.vector.tensor_mul(y, y, t2)

                # apply rstd with one stride-0-broadcast multiply (f32 -> bf16)
                qkb = qkpool.tile([128, QK], BF16, tag="qkb")
                nc.vector.tensor_tensor(
                    qkb.rearrange("p (g d) -> p g d", d=HD),
                    qk_sb.rearrange("p (g d) -> p g d", d=HD),
                    bcast_inner(y, HD), op=MUL,
                )

                # transpose normed qk -> qT/zkT, folding in the norm weights
                for half in range(2):
                    p_tr = ptr.tile([128, 4, 128], BF16, tag="ptr")
                    for j in range(2):
                        cb = half * 2 + j
                        nc.tensor.matmul(
                            p_tr[:, j, :], qkb[:, cb * 128:(cb + 1) * 128], ident,
                            is_transpose=True, start=(j == 0), stop=(j == 1),
                        )
                    for j in range(2):
                        cb = half * 2 + j
                        if cb < 2:
                            nc.vector.tensor_scalar_mul(qT[:, cb, ts], p_tr[:, j, :],
                                                        normw_sb[:, cb:cb + 1])
                        else:
                            kb = cb - 2    # head pair block
                            for par in range(2):
                                rows = slice(par * 64, par * 64 + 64)
                                nc.scalar.activation(
                                    zkT[rows, 2 * kb + par, t, :], p_tr[rows, j, :],
                                    AF.Copy, scale=normw_sb[rows, cb:cb + 1],
                                )

        # ---------------- phase B: attention (+ projection, shared pools) ----
        with ExitStack() as bctx:
            spool = bctx.enter_context(tc.tile_pool(name="ps", bufs=2, space="PSUM"))
            opool = bctx.enter_context(tc.tile_pool(name="po", bufs=1, space="PSUM"))
            ppool = bctx.enter_context(tc.tile_pool(name="pp", bufs=2, space="PSUM"))
            ptpool = bctx.enter_context(tc.tile_pool(name="pt", bufs=3))
            rpool = bctx.enter_context(tc.tile_pool(name="rec", bufs=2))
            outpool = bctx.enter_context(tc.tile_pool(name="outsb", bufs=4))

            for h in range(HPC):
                cb = h // 2                # column block in qT / pair block in aT
                # even head: o_un^T on rows 0:64, denom on 64:128; odd: swapped
                osl = slice(0, 64) if h % 2 == 0 else slice(64, 128)
                dsl = slice(64, 128) if h % 2 == 0 else slice(0, 64)
                for qh in range(2):        # 1024-token halves of the query axis
                    po = opool.tile([128, 2, 512], F32, tag="po")
                    prev = None
                    for kt in range(NT):
                        ps = spool.tile([128, 1024], F32, tag="ps")
                        for i in range(2):
                            qsl = slice(qh * 1024 + i * 512, qh * 1024 + (i + 1) * 512)
                            nc.tensor.matmul(
                                ps[:, i * 512:(i + 1) * 512],
                                zkT[:, h, kt, :],
                                qT[:, cb, qsl],
                                start=True, stop=True,
                            )
                        pt = ptpool.tile([128, 1024], BF16, tag="pt")
                        nc.scalar.activation(pt, ps, AF.Exp, scale=0.125)
                        if prev is not None:
                            ppt, pkt = prev
                            for i in range(2):
                                nc.tensor.matmul(
                                    po[:, i, :], vaug[:, pkt, h, :],
                                    ppt[:, i * 512:(i + 1) * 512],
                                    start=(pkt == 0), stop=False,
                                )
                        prev = (pt, kt)
                    ppt, pkt = prev
                    for i in range(2):
                        nc.tensor.matmul(
                            po[:, i, :], vaug[:, pkt, h, :],
                            ppt[:, i * 512:(i + 1) * 512],
                            start=False, stop=True,
                        )
                    # normalize: aT rows osl = po rows osl * recip(po rows dsl)
                    rec = rpool.tile([128, 2, 512], F32, tag="rec")
                    nc.vector.reciprocal_approx_fast(rec, po)
                    nc.vector.tensor_mul(
                        aT[osl, cb, qh * 1024:(qh + 1) * 1024]
                          .rearrange("p (i q) -> p i q", i=2),
                        po[osl, :, :],
                        rec[dsl, :, :],
                    )

            # ---------------- phase C: partial projection ----------------
            for t in range(NT):
                ts = slice(t * 128, (t + 1) * 128)
                for jg in range(2):
                    pp = ppool.tile([128, 512], F32, tag="pp")
                    for hb in range(2):
                        nc.tensor.matmul(
                            pp, aT[:, hb, ts],
                            wproj_sb[:, hb, jg * 512:(jg + 1) * 512],
                            start=(hb == 0), stop=(hb == 1),
                        )
                    outsb = outpool.tile([128, 512], F32, tag="outsb")
                    nc.vector.tensor_copy(outsb, pp)
                    nc.sync.dma_start(
                        out=out_ext[ts, jg * 512:(jg + 1) * 512], in_=outsb
                    )

        if dbg:
            with tc.tile_pool(name="dbgp", bufs=1) as dp:
                for nm, dst, src in (("q", dbg_q, qT), ("a", dbg_a, aT)):
                    f = dp.tile([128, 2, N], F32, tag=f"f{nm}", name=f"f{nm}")
                    nc.vector.tensor_copy(f, src)
                    nc.sync.dma_start(out=dst[:, :, :], in_=f)
                fk = dp.tile([128, HPC, NT, 128], F32, tag="fk", name="fk")
                nc.vector.tensor_copy(fk, zkT)
                nc.sync.dma_start(out=dbg_k[:, :, :, :], in_=fk)
                fv = dp.tile([128, NT, HPC, 128], F32, tag="fv", name="fv")
                nc.vector.tensor_copy(fv, vaug)
                nc.sync.dma_start(out=dbg_v[:, :, :, :], in_=fv)

    nc.finalize()
    return nc


def make_in_maps(x, qkv_w, qkv_b, q_norm_w, k_norm_w, proj_w, proj_b):
    """Shard the full inputs into the 8 per-core input maps."""
    bf = ml_dtypes.bfloat16
    in_maps = []
    for c in range(8):
        b, g = c // 4, c % 4
        ch = np.arange(4 * g * HD, 4 * (g + 1) * HD)          # this core's head channels
        wqkv_c = np.concatenate(
            [qkv_w[:, ch], qkv_w[:, C + ch], qkv_w[:, 2 * C + ch]], axis=1
        )
        bqkv_c = np.concatenate([qkv_b[ch], qkv_b[C + ch], qkv_b[2 * C + ch]])
        normw = np.concatenate([np.tile(q_norm_w, HPC), np.tile(k_norm_w, HPC)])
        # wproj rows for this core as [128 rows of head-pair, pair, C]
        wproj_c = proj_w[ch, :].reshape(2, V // 2, C).transpose(1, 0, 2)
        in_maps.append({
            "x": np.ascontiguousarray(x[b]).astype(bf),
            "wqkv": np.ascontiguousarray(wqkv_c).astype(bf),
            "bqkv": np.ascontiguousarray(bqkv_c, np.float32),
            "normw": np.ascontiguousarray(normw, np.float32),
            "wproj": np.ascontiguousarray(wproj_c).astype(bf),
        })
    return in_maps


_NC_CACHE = []


def kernel(x, qkv_w, qkv_b, q_norm_w, k_norm_w, proj_w, proj_b,
           _run_kwargs=None, _res_box=None):
    x = np.asarray(x); qkv_w = np.asarray(qkv_w); qkv_b = np.asarray(qkv_b)
    q_norm_w = np.asarray(q_norm_w); k_norm_w = np.asarray(k_norm_w)
    proj_w = np.asarray(proj_w); proj_b = np.asarray(proj_b)

    if not _NC_CACHE:
        _NC_CACHE.append(build_nc())
    nc = _NC_CACHE[0]
    in_maps = make_in_maps(x, qkv_w, qkv_b, q_norm_w, k_norm_w, proj_w, proj_b)
    res = run_bass_kernel_spmd(nc, in_maps, core_ids=list(range(8)),
                               **(_run_kwargs or {}))
    if _res_box is not None:
        _res_box["res"] = res
    out = np.zeros((B, N, C), np.float32)
    for c in range(8):
        out[c // 4] += res.results[c]["out"]
    out += proj_b[None, None, :].astype(np.float32)
    return out


if __name__ == "__main__":
    rng = np.random.default_rng(0)
    x = rng.standard_normal((B, N, C)).astype(np.float32)
    qkv_w = (rng.standard_normal((C, 3 * C)) / np.sqrt(C)).astype(np.float32)
    qkv_b = np.zeros((3 * C,), np.float32)
    qn = np.ones((HD,), np.float32)
    kn = np.ones((HD,), np.float32)
    proj_w = (rng.standard_normal((C, C)) / np.sqrt(C)).astype(np.float32)
    proj_b = np.zeros((C,), np.float32)
    out = kernel(x, qkv_w, qkv_b, qn, kn, proj_w, proj_b)
    print("out", out.shape, out.dtype, float(np.abs(out).mean()))


# revision 30
# speedup vs baseline: 1.3839x; 1.0045x over previous
"""Fused multi-head attention block on 8 TRN2 NeuronCores.

reference: qkv = x@Wqkv+b; q,k rmsnorm'd per head; softmax(q k^T/sqrt(hd)) v; proj.
Shapes: x [2,2048,1024], H=16 heads, hd=64.

Distribution (no collectives): 8 cores = 2 batches x 4 head-groups (4 heads each).
Core c: batch b=c//4, heads 4g..4g+3 (g=c%4). Each core computes the partial
projection output (proj_w row-sharded over its heads) for its batch; the host
sums the 4 partials per batch and adds proj_b.

Per-core pipeline (bf16 matmul operands, f32 PSUM accumulation):
  A) x (bf16, host-converted) -> PE-transpose (4 packed per PSUM bank) -> xT;
     qkv GEMM; rmsnorm(q,k) with rstd on DVE only (seed (3-m)/2 + Newton);
     one stride-0-broadcast multiply applies rstd; PE-transpose normed qk ->
     qT,kT bf16 (norm weights folded into the evict); v (+bias) -> vaug.
  B) per (head, qtok-half 1024): software-pipelined over ktiles: scores
     S^T(kt) = kT^T qT (2 MMs), exp(kt) on ACT (scale=1/8, no max subtract,
     scores in [-6,6]), AV(kt-1) accumulates with the REUSED stationary
     vaug = [v|ones] (even heads) / [ones|v] (odd heads): out rows give the
     unnormalized out^T on one 64-partition half and the broadcast softmax
     denominator on the other. Epilogue: aT = out^T * approx-recip(denom).
     Head parity places outputs so aT packs head pairs on full 128 partitions.
  C) partial projection straight from aT (2 MMs, K=128), DMA out.
"""

from contextlib import ExitStack

import ml_dtypes
import numpy as np

import concourse.bass as bass
import concourse.mybir as mybir
import concourse.tile as tile
from concourse import bacc
from concourse.bass_utils import run_bass_kernel_spmd
from concourse.masks import make_identity

B, N, C = 2, 2048, 1024
H, HD = 16, 64
HPC = 4                 # heads per core
NT = N // 128           # 16 token tiles
KT8 = C // 128          # 8 contraction tiles for the qkv GEMM
QK = 2 * HPC * HD       # 512 qk channels per core
V = HPC * HD            # 256 v channels per core
EPS = 1e-6
F32 = mybir.dt.float32
BF16 = mybir.dt.bfloat16
AF = mybir.ActivationFunctionType
MUL = mybir.AluOpType.mult
ADD = mybir.AluOpType.add


def bcast_inner(ap, n):
    """Append a stride-0 inner dim of size n to a 2D AP."""
    return bass.AP(tensor=ap.tensor, offset=ap.offset,
                   ap=[list(ap.ap[0]), list(ap.ap[1]), [0, n]])


def build_nc(dbg=False):
    nc = bacc.Bacc("TRN2", target_bir_lowering=False, debug=False)

    x_ext = nc.declare_dram_parameter("x", [N, C], BF16, isOutput=False)
    wqkv_ext = nc.declare_dram_parameter("wqkv", [C, QK + V], BF16, isOutput=False)
    bqkv_ext = nc.declare_dram_parameter("bqkv", [QK + V], F32, isOutput=False)
    normw_ext = nc.declare_dram_parameter("normw", [QK], F32, isOutput=False)
    wproj_ext = nc.declare_dram_parameter("wproj", [V // 2, 2, C], BF16, isOutput=False)
    out_ext = nc.declare_dram_parameter("out", [N, C], F32, isOutput=True)
    if dbg:
        dbg_q = nc.declare_dram_parameter("dbg_q", [128, 2, N], F32, isOutput=True)
        dbg_k = nc.declare_dram_parameter("dbg_k", [128, HPC, NT, 128], F32, isOutput=True)
        dbg_v = nc.declare_dram_parameter("dbg_v", [128, NT, HPC, 128], F32, isOutput=True)
        dbg_a = nc.declare_dram_parameter("dbg_a", [128, 2, N], F32, isOutput=True)

    with tile.TileContext(nc) as tc, ExitStack() as ctx:
        singles = ctx.enter_context(tc.tile_pool(name="singles", bufs=1))

        ident = singles.tile([128, 128], BF16, tag="ident")
        make_identity(nc, ident)

        wqkv_sb = singles.tile([128, KT8, QK + V], BF16, tag="wqkv")
        for kt in range(KT8):
            nc.sync.dma_start(
                out=wqkv_sb[:, kt, :], in_=wqkv_ext[kt * 128:(kt + 1) * 128, :]
            )
        wproj_sb = singles.tile([128, 2, C], BF16, tag="wproj")
        nc.sync.dma_start(out=wproj_sb, in_=wproj_ext[:, :, :])
        bias_sb = singles.tile([128, QK + V], F32, tag="bias")
        nc.sync.dma_start(out=bias_sb, in_=bqkv_ext[:].partition_broadcast(128))
        # normw_sb[p, cb] = normw[cb*128 + p]
        normw_sb = singles.tile([128, 4], F32, tag="normw")
        nc.sync.dma_start(out=normw_sb, in_=normw_ext[:].rearrange("(b p) -> p b", p=128))

        # persistent activations
        qT = singles.tile([128, 2, N], BF16, tag="qT")     # channel-major q
        # k^T stored per head, zero-padded to K=128 on the partition axis:
        # head h occupies rows (h%2)*64..+64 of [:, h, kt, :]; other rows stay
        # zero so the scores matmul runs with a full-K stationary (K=64
        # matmuls execute at half rate and keep the PE clock cold).
        zkT = singles.tile([128, HPC, NT, 128], BF16, tag="zkT")
        nc.vector.memset(zkT, 0.0)
        # AV stationary blocks: even head s: [v_s | ones]; odd: [ones | v_s]
        vaug = singles.tile([128, NT, HPC, 128], BF16, tag="vaug")
        nc.vector.memset(vaug, 1.0)
        # normalized attention output, transposed; block hb packs heads 2hb,2hb+1
        aT = singles.tile([128, 2, N], BF16, tag="aT")

        # ---------------- phase A: qkv + rmsnorm + transposes ----------------
        with ExitStack() as actx:
            xpool = actx.enter_context(tc.tile_pool(name="xin", bufs=NT))
            xtpool = actx.enter_context(tc.tile_pool(name="xt", bufs=3))
            qkpool = actx.enter_context(tc.tile_pool(name="qksb", bufs=4))
            stpool = actx.enter_context(tc.tile_pool(name="stats", bufs=8))
            ptr = actx.enter_context(tc.tile_pool(name="ptr", bufs=3, space="PSUM"))
            pqk = actx.enter_context(tc.tile_pool(name="pqk", bufs=2, space="PSUM"))
            pv = actx.enter_context(tc.tile_pool(name="pv", bufs=2, space="PSUM"))

            # prefetch every x tile up front so the DMA queues run ahead
            xins = []
            for t in range(NT):
                xin = xpool.tile([128, C], BF16, tag="xin", name=f"xin{t}")
                nc.sync.dma_start(out=xin, in_=x_ext[t * 128:(t + 1) * 128, :])
                xins.append(xin)

            for t in range(NT):
                ts = slice(t * 128, (t + 1) * 128)
                xin = xins[t]

                # transpose x tile -> xT blocks [C-part, tok]; 4 per PSUM bank,
                # evicted with one ScalarE copy each
                xt = xtpool.tile([128, KT8, 128], BF16, tag="xt")
                for half in range(2):
                    p_tr = ptr.tile([128, 4, 128], BF16, tag="ptr")
                    for j in range(4):
                        kt = half * 4 + j
                        nc.tensor.matmul(
                            p_tr[:, j, :], xin[:, kt * 128:(kt + 1) * 128], ident,
                            is_transpose=True, start=(j == 0), stop=(j == 3),
                        )
                    nc.scalar.copy(xt[:, half * 4:half * 4 + 4, :], p_tr)

                # qkv GEMM for this token tile
                p_qk = pqk.tile([128, QK], F32, tag="pqk")
                p_v = pv.tile([128, V], F32, tag="pv")
                for kt in range(KT8):
                    nc.tensor.matmul(
                        p_qk, xt[:, kt, :], wqkv_sb[:, kt, 0:QK],
                        start=(kt == 0), stop=(kt == KT8 - 1),
                    )
                for kt in range(KT8):
                    nc.tensor.matmul(
                        p_v, xt[:, kt, :], wqkv_sb[:, kt, QK:QK + V],
                        start=(kt == 0), stop=(kt == KT8 - 1),
                    )

                # evictions with bias add; v goes to its parity-dependent half
                qk_sb = qkpool.tile([128, QK], F32, tag="qksb")
                nc.vector.tensor_add(qk_sb, p_qk, bias_sb[:, 0:QK])
                pv3 = p_v.rearrange("p (h d) -> p h d", d=HD)
                bv3 = bias_sb[:, QK:QK + V].rearrange("p (h d) -> p h d", d=HD)
                nc.vector.tensor_add(vaug[:, t, 0::2, 0:HD], pv3[:, 0::2, :],
                                     bv3[:, 0::2, :])
                nc.vector.tensor_add(vaug[:, t, 1::2, HD:128], pv3[:, 1::2, :],
                                     bv3[:, 1::2, :])

                # rmsnorm: rstd = 1/sqrt(m), m = ssq/64 + eps:
                # seed (3-m)/2 then 2 Newton steps y *= 1.5 - 0.5*m*y^2
                sq = qkpool.tile([128, QK], F32, tag="sq")
                nc.scalar.square(sq, qk_sb)
                m = stpool.tile([128, 2 * HPC], F32, tag="m")
                nc.vector.tensor_reduce(
                    m, sq.rearrange("p (g d) -> p g d", d=HD),
                    axis=mybir.AxisListType.X, op=mybir.AluOpType.add,
                )
                nc.vector.tensor_scalar(m, m, 1.0 / HD, EPS, op0=MUL, op1=ADD)
                y = stpool.tile([128, 2 * HPC], F32, tag="y")
                nc.vector.tensor_scalar(y, m, -0.5, 1.5, op0=MUL, op1=ADD)
                t2 = stpool.tile([128, 2 * HPC], F32, tag="t2")
                for _ in range(2):
                    nc.vector.tensor_mul(t2, y, y)
                    nc.vector.tensor_mul(t2, t2, m)
                    nc.vector.tensor_scalar(t2, t2, -0.5, 1.5, op0=MUL, op1=ADD)
                    nc.vector.tensor_mul(y, y, t2)

                # apply rstd with one stride-0-broadcast multiply (f32 -> bf16)
                qkb = qkpool.tile([128, QK], BF16, tag="qkb")
                nc.vector.tensor_tensor(
                    qkb.rearrange("p (g d) -> p g d", d=HD),
                    qk_sb.rearrange("p (g d) -> p g d", d=HD),
                    bcast_inner(y, HD), op=MUL,
                )

                # transpose normed qk -> qT/zkT, folding in the norm weights
                for half in range(2):
                    p_tr = ptr.tile([128, 4, 128], BF16, tag="ptr")
                    for j in range(2):
                        cb = half * 2 + j
                        nc.tensor.matmul(
                            p_tr[:, j, :], qkb[:, cb * 128:(cb + 1) * 128], ident,
                            is_transpose=True, start=(j == 0), stop=(j == 1),
                        )
                    for j in range(2):
                        cb = half * 2 + j
                        if cb < 2:
                            nc.vector.tensor_scalar_mul(qT[:, cb, ts], p_tr[:, j, :],
                                                        normw_sb[:, cb:cb + 1])
                        else:
                            kb = cb - 2    # head pair block
                            for par in range(2):
                                rows = slice(par * 64, par * 64 + 64)
                                nc.scalar.activation(
                                    zkT[rows, 2 * kb + par, t, :], p_tr[rows, j, :],
                                    AF.Copy, scale=normw_sb[rows, cb:cb + 1],
                                )

        # ---------------- phase B: attention (+ projection, shared pools) ----
        with ExitStack() as bctx:
            spool = bctx.enter_context(tc.tile_pool(name="ps", bufs=2, space="PSUM"))
            opool = bctx.enter_context(tc.tile_pool(name="po", bufs=1, space="PSUM"))
            ppool = bctx.enter_context(tc.tile_pool(name="pp", bufs=2, space="PSUM"))
            ptpool = bctx.enter_context(tc.tile_pool(name="pt", bufs=3))
            rpool = bctx.enter_context(tc.tile_pool(name="rec", bufs=2))
            outpool = bctx.enter_context(tc.tile_pool(name="outsb", bufs=4))

            for qh in range(2):            # 1024-token halves of the query axis
                for h in range(HPC):
                    cb = h // 2            # column block in qT / pair block in aT
                    # even head: o_un^T rows 0:64, denom rows 64:128; odd: swapped
                    osl = slice(0, 64) if h % 2 == 0 else slice(64, 128)
                    dsl = slice(64, 128) if h % 2 == 0 else slice(0, 64)
                    po = opool.tile([128, 2, 512], F32, tag="po")
                    prev = None
                    for kt in range(NT):
                        ps = spool.tile([128, 1024], F32, tag="ps")
                        for i in range(2):
                            qsl = slice(qh * 1024 + i * 512, qh * 1024 + (i + 1) * 512)
                            nc.tensor.matmul(
                                ps[:, i * 512:(i + 1) * 512],
                                zkT[:, h, kt, :],
                                qT[:, cb, qsl],
                                start=True, stop=True,
                            )
                        pt = ptpool.tile([128, 1024], BF16, tag="pt")
                        nc.scalar.activation(pt, ps, AF.Exp, scale=0.125)
                        if prev is not None:
                            ppt, pkt = prev
                            for i in range(2):
                                nc.tensor.matmul(
                                    po[:, i, :], vaug[:, pkt, h, :],
                                    ppt[:, i * 512:(i + 1) * 512],
                                    start=(pkt == 0), stop=False,
                                )
                        prev = (pt, kt)
                    ppt, pkt = prev
                    for i in range(2):
                        nc.tensor.matmul(
                            po[:, i, :], vaug[:, pkt, h, :],
                            ppt[:, i * 512:(i + 1) * 512],
                            start=False, stop=True,
                        )
                    # normalize: aT rows osl = po rows osl * recip(po rows dsl)
                    rec = rpool.tile([128, 2, 512], F32, tag="rec")
                    nc.vector.reciprocal_approx_fast(rec, po)
                    nc.vector.tensor_mul(
                        aT[osl, cb, qh * 1024:(qh + 1) * 1024]
                          .rearrange("p (i q) -> p i q", i=2),
                        po[osl, :, :],
                        rec[dsl, :, :],
                    )

                # ---- projection for this query half (overlaps next half) ----
                for t in range(qh * NT // 2, (qh + 1) * NT // 2):
                    ts = slice(t * 128, (t + 1) * 128)
                    for jg in range(2):
                        pp = ppool.tile([128, 512], F32, tag="pp")
                        for hb in range(2):
                            nc.tensor.matmul(
                                pp, aT[:, hb, ts],
                                wproj_sb[:, hb, jg * 512:(jg + 1) * 512],
                                start=(hb == 0), stop=(hb == 1),
                            )
                        outsb = outpool.tile([128, 512], F32, tag="outsb")
                        nc.vector.tensor_copy(outsb, pp)
                        nc.sync.dma_start(
                            out=out_ext[ts, jg * 512:(jg + 1) * 512], in_=outsb
                        )

        if dbg:
            with tc.tile_pool(name="dbgp", bufs=1) as dp:
                for nm, dst, src in (("q", dbg_q, qT), ("a", dbg_a, aT)):
                    f = dp.tile([128, 2, N], F32, tag=f"f{nm}", name=f"f{nm}")
                    nc.vector.tensor_copy(f, src)
                    nc.sync.dma_start(out=dst[:, :, :], in_=f)
                fk = dp.tile([128, HPC, NT, 128], F32, tag="fk", name="fk")
                nc.vector.tensor_copy(fk, zkT)
                nc.sync.dma_start(out=dbg_k[:, :, :, :], in_=fk)
                fv = dp.tile([128, NT, HPC, 128], F32, tag="fv", name="fv")
                nc.vector.tensor_copy(fv, vaug)
                nc.sync.dma_start(out=dbg_v[:, :, :, :], in_=fv)

    nc.finalize()
    return nc


def make_in_maps(x, qkv_w, qkv_b, q_norm_w, k_norm_w, proj_w, proj_b):
    """Shard the full inputs into the 8 per-core input maps."""
    bf = ml_dtypes.bfloat16
    in_maps = []
    for c in range(8):
        b, g = c // 4, c % 4
        ch = np.arange(4 * g * HD, 4 * (g + 1) * HD)          # this core's head channels
        wqkv_c = np.concatenate(
            [qkv_w[:, ch], qkv_w[:, C + ch], qkv_w[:, 2 * C + ch]], axis=1
        )
        bqkv_c = np.concatenate([qkv_b[ch], qkv_b[C + ch], qkv_b[2 * C + ch]])
        normw = np.concatenate([np.tile(q_norm_w, HPC), np.tile(k_norm_w, HPC)])
        # wproj rows for this core as [128 rows of head-pair, pair, C]
        wproj_c = proj_w[ch, :].reshape(2, V // 2, C).transpose(1, 0, 2)
        in_maps.append({
            "x": np.ascontiguousarray(x[b]).astype(bf),
            "wqkv": np.ascontiguousarray(wqkv_c).astype(bf),
            "bqkv": np.ascontiguousarray(bqkv_c, np.float32),
            "normw": np.ascontiguousarray(normw, np.float32),
            "wproj": np.ascontiguousarray(wproj_c).astype(bf),
        })
    return in_maps


_NC_CACHE = []


def kernel(x, qkv_w, qkv_b, q_norm_w, k_norm_w, proj_w, proj_b,
           _run_kwargs=None, _res_box=None):
    x = np.asarray(x); qkv_w = np.asarray(qkv_w); qkv_b = np.asarray(qkv_b)
    q_norm_w = np.asarray(q_norm_w); k_norm_w = np.asarray(k_norm_w)
    proj_w = np.asarray(proj_w); proj_b = np.asarray(proj_b)

    if not _NC_CACHE:
        _NC_CACHE.append(build_nc())
    nc = _NC_CACHE[0]
    in_maps = make_in_maps(x, qkv_w, qkv_b, q_norm_w, k_norm_w, proj_w, proj_b)
    res = run_bass_kernel_spmd(nc, in_maps, core_ids=list(range(8)),
                               **(_run_kwargs or {}))
    if _res_box is not None:
        _res_box["res"] = res
    out = np.zeros((B, N, C), np.float32)
    for c in range(8):
        out[c // 4] += res.results[c]["out"]
    out += proj_b[None, None, :].astype(np.float32)
    return out


if __name__ == "__main__":
    rng = np.random.default_rng(0)
    x = rng.standard_normal((B, N, C)).astype(np.float32)
    qkv_w = (rng.standard_normal((C, 3 * C)) / np.sqrt(C)).astype(np.float32)
    qkv_b = np.zeros((3 * C,), np.float32)
    qn = np.ones((HD,), np.float32)
    kn = np.ones((HD,), np.float32)
    proj_w = (rng.standard_normal((C, C)) / np.sqrt(C)).astype(np.float32)
    proj_b = np.zeros((C,), np.float32)
    out = kernel(x, qkv_w, qkv_b, qn, kn, proj_w, proj_b)
    print("out", out.shape, out.dtype, float(np.abs(out).mean()))


# revision 33
# speedup vs baseline: 1.8058x; 1.3049x over previous
"""Fused multi-head attention block on 8 TRN2 NeuronCores.

reference: qkv = x@Wqkv+b; q,k rmsnorm'd per head; softmax(q k^T/sqrt(hd)) v; proj.
Shapes: x [2,2048,1024], H=16 heads, hd=64.

Distribution (no collectives): 8 cores = 2 batches x 4 head-groups (4 heads each).
Core c: batch b=c//4, heads 4g..4g+3 (g=c%4). Each core computes the partial
projection output (proj_w row-sharded over its heads) for its batch; the host
sums the 4 partials per batch and adds proj_b.

Per-core pipeline (bf16 matmul operands, f32 PSUM accumulation):
  A) x (bf16, host-converted) -> PE-transpose (4 packed per PSUM bank) -> xT;
     qkv GEMM; rmsnorm(q,k) with rstd on DVE only (seed (3-m)/2 + Newton);
     one stride-0-broadcast multiply applies rstd; PE-transpose normed qk ->
     qT,kT bf16 (norm weights folded into the evict); v (+bias) -> vaug.
  B) per (head, qtok-half 1024): software-pipelined over ktiles: scores
     S^T(kt) = kT^T qT (2 MMs), exp(kt) on ACT (scale=1/8, no max subtract,
     scores in [-6,6]), AV(kt-1) accumulates with the REUSED stationary
     vaug = [v|ones] (even heads) / [ones|v] (odd heads): out rows give the
     unnormalized out^T on one 64-partition half and the broadcast softmax
     denominator on the other. Epilogue: aT = out^T * approx-recip(denom).
     Head parity places outputs so aT packs head pairs on full 128 partitions.
  C) partial projection straight from aT (2 MMs, K=128), DMA out.
"""

from contextlib import ExitStack

import ml_dtypes
import numpy as np

import concourse.bass as bass
import concourse.mybir as mybir
import concourse.tile as tile
from concourse import bacc
from concourse.bass_utils import run_bass_kernel_spmd
from concourse.masks import make_identity

B, N, C = 2, 2048, 1024
H, HD = 16, 64
HPC = 4                 # heads per core
NT = N // 128           # 16 token tiles
KT8 = C // 128          # 8 contraction tiles for the qkv GEMM
QK = 2 * HPC * HD       # 512 qk channels per core
V = HPC * HD            # 256 v channels per core
EPS = 1e-6
F32 = mybir.dt.float32
BF16 = mybir.dt.bfloat16
AF = mybir.ActivationFunctionType
MUL = mybir.AluOpType.mult
ADD = mybir.AluOpType.add


def bcast_inner(ap, n):
    """Append a stride-0 inner dim of size n to a 2D AP."""
    return bass.AP(tensor=ap.tensor, offset=ap.offset,
                   ap=[list(ap.ap[0]), list(ap.ap[1]), [0, n]])


def build_nc(dbg=False):
    nc = bacc.Bacc("TRN2", target_bir_lowering=False, debug=False)

    x_ext = nc.declare_dram_parameter("x", [N, C], BF16, isOutput=False)
    wqkv_ext = nc.declare_dram_parameter("wqkv", [C, QK + V], BF16, isOutput=False)
    bqkv_ext = nc.declare_dram_parameter("bqkv", [QK + V], F32, isOutput=False)
    normw_ext = nc.declare_dram_parameter("normw", [QK], F32, isOutput=False)
    wproj_ext = nc.declare_dram_parameter("wproj", [V // 2, 2, C], BF16, isOutput=False)
    out_ext = nc.declare_dram_parameter("out", [N, C], F32, isOutput=True)
    if dbg:
        dbg_q = nc.declare_dram_parameter("dbg_q", [128, 2, N], F32, isOutput=True)
        dbg_k = nc.declare_dram_parameter("dbg_k", [128, HPC, NT, 128], F32, isOutput=True)
        dbg_v = nc.declare_dram_parameter("dbg_v", [128, NT, HPC, 128], F32, isOutput=True)
        dbg_a = nc.declare_dram_parameter("dbg_a", [128, 2, N], F32, isOutput=True)

    with tile.TileContext(nc) as tc, ExitStack() as ctx:
        singles = ctx.enter_context(tc.tile_pool(name="singles", bufs=1))

        ident = singles.tile([128, 128], BF16, tag="ident")
        make_identity(nc, ident)

        wqkv_sb = singles.tile([128, KT8, QK + V], BF16, tag="wqkv")
        for kt in range(KT8):
            nc.sync.dma_start(
                out=wqkv_sb[:, kt, :], in_=wqkv_ext[kt * 128:(kt + 1) * 128, :]
            )
        wproj_sb = singles.tile([128, 2, C], BF16, tag="wproj")
        nc.sync.dma_start(out=wproj_sb, in_=wproj_ext[:, :, :])
        bias_sb = singles.tile([128, QK + V], F32, tag="bias")
        nc.sync.dma_start(out=bias_sb, in_=bqkv_ext[:].partition_broadcast(128))
        # normw_sb[p, cb] = normw[cb*128 + p]
        normw_sb = singles.tile([128, 4], F32, tag="normw")
        nc.sync.dma_start(out=normw_sb, in_=normw_ext[:].rearrange("(b p) -> p b", p=128))

        # persistent activations
        qT = singles.tile([128, 2, N], BF16, tag="qT")     # channel-major q
        # k^T stored per head, zero-padded to K=128 on the partition axis:
        # head h occupies rows (h%2)*64..+64 of [:, h, kt, :]; other rows stay
        # zero so the scores matmul runs with a full-K stationary (K=64
        # matmuls execute at half rate and keep the PE clock cold).
        zkT = singles.tile([128, HPC, NT, 128], BF16, tag="zkT")
        nc.vector.memset(zkT, 0.0)
        # AV stationary blocks: even head s: [v_s | ones]; odd: [ones | v_s]
        vaug = singles.tile([128, NT, HPC, 128], BF16, tag="vaug")
        nc.vector.memset(vaug, 1.0)
        # normalized attention output, transposed; block hb packs heads 2hb,2hb+1
        aT = singles.tile([128, 2, N], BF16, tag="aT")

        # ---------------- phase A: qkv + rmsnorm + transposes ----------------
        with ExitStack() as actx:
            xpool = actx.enter_context(tc.tile_pool(name="xin", bufs=NT))
            xtpool = actx.enter_context(tc.tile_pool(name="xt", bufs=3))
            qkpool = actx.enter_context(tc.tile_pool(name="qksb", bufs=4))
            stpool = actx.enter_context(tc.tile_pool(name="stats", bufs=8))
            ptr = actx.enter_context(tc.tile_pool(name="ptr", bufs=3, space="PSUM"))
            pqk = actx.enter_context(tc.tile_pool(name="pqk", bufs=2, space="PSUM"))
            pv = actx.enter_context(tc.tile_pool(name="pv", bufs=2, space="PSUM"))

            # prefetch every x tile up front so the DMA queues run ahead
            xins = []
            for t in range(NT):
                xin = xpool.tile([128, C], BF16, tag="xin", name=f"xin{t}")
                nc.sync.dma_start(out=xin, in_=x_ext[t * 128:(t + 1) * 128, :])
                xins.append(xin)

            def qk_transposes(qkb, t):
                """Transpose normed qk of tile t into qT/zkT (norm w folded)."""
                ts = slice(t * 128, (t + 1) * 128)
                for half in range(2):
                    p_tr = ptr.tile([128, 4, 128], BF16, tag="ptr")
                    for j in range(2):
                        cb = half * 2 + j
                        nc.tensor.matmul(
                            p_tr[:, j, :], qkb[:, cb * 128:(cb + 1) * 128], ident,
                            is_transpose=True, start=(j == 0), stop=(j == 1),
                        )
                    for j in range(2):
                        cb = half * 2 + j
                        if cb < 2:
                            nc.vector.tensor_scalar_mul(qT[:, cb, ts], p_tr[:, j, :],
                                                        normw_sb[:, cb:cb + 1])
                        else:
                            kb = cb - 2    # head pair block
                            for par in range(2):
                                rows = slice(par * 64, par * 64 + 64)
                                if par == 0:
                                    nc.vector.tensor_scalar_mul(
                                        zkT[rows, 2 * kb, t, :], p_tr[rows, j, :],
                                        normw_sb[rows, cb:cb + 1])
                                else:
                                    nc.scalar.activation(
                                        zkT[rows, 2 * kb + par, t, :],
                                        p_tr[rows, j, :],
                                        AF.Copy, scale=normw_sb[rows, cb:cb + 1],
                                    )

            qkb_prev = None
            for t in range(NT):
                ts = slice(t * 128, (t + 1) * 128)
                xin = xins[t]

                # transpose x tile -> xT blocks [C-part, tok]; 4 per PSUM bank,
                # evicted with one ScalarE copy each
                xt = xtpool.tile([128, KT8, 128], BF16, tag="xt")
                for half in range(2):
                    p_tr = ptr.tile([128, 4, 128], BF16, tag="ptr")
                    for j in range(4):
                        kt = half * 4 + j
                        nc.tensor.matmul(
                            p_tr[:, j, :], xin[:, kt * 128:(kt + 1) * 128], ident,
                            is_transpose=True, start=(j == 0), stop=(j == 3),
                        )
                    nc.scalar.copy(xt[:, half * 4:half * 4 + 4, :], p_tr)

                # qkv GEMM for this token tile
                p_qk = pqk.tile([128, QK], F32, tag="pqk")
                p_v = pv.tile([128, V], F32, tag="pv")
                for kt in range(KT8):
                    nc.tensor.matmul(
                        p_qk, xt[:, kt, :], wqkv_sb[:, kt, 0:QK],
                        start=(kt == 0), stop=(kt == KT8 - 1),
                    )
                for kt in range(KT8):
                    nc.tensor.matmul(
                        p_v, xt[:, kt, :], wqkv_sb[:, kt, QK:QK + V],
                        start=(kt == 0), stop=(kt == KT8 - 1),
                    )

                # previous tile's qk transposes go here so the PE never waits
                # on this tile's rmsnorm DVE chain (in-order queue)
                if qkb_prev is not None:
                    qk_transposes(*qkb_prev)

                # evictions with bias add; v goes to its parity-dependent half
                qk_sb = qkpool.tile([128, QK], F32, tag="qksb")
                nc.vector.tensor_add(qk_sb, p_qk, bias_sb[:, 0:QK])
                pv3 = p_v.rearrange("p (h d) -> p h d", d=HD)
                bv3 = bias_sb[:, QK:QK + V].rearrange("p (h d) -> p h d", d=HD)
                nc.vector.tensor_add(vaug[:, t, 0::2, 0:HD], pv3[:, 0::2, :],
                                     bv3[:, 0::2, :])
                nc.vector.tensor_add(vaug[:, t, 1::2, HD:128], pv3[:, 1::2, :],
                                     bv3[:, 1::2, :])

                # rmsnorm: rstd = 1/sqrt(m), m = ssq/64 + eps:
                # seed (3-m)/2 then 2 Newton steps y *= 1.5 - 0.5*m*y^2
                sq = qkpool.tile([128, QK], F32, tag="sq")
                nc.scalar.square(sq, qk_sb)
                m = stpool.tile([128, 2 * HPC], F32, tag="m")
                nc.vector.tensor_reduce(
                    m, sq.rearrange("p (g d) -> p g d", d=HD),
                    axis=mybir.AxisListType.X, op=mybir.AluOpType.add,
                )
                nc.vector.tensor_scalar(m, m, 1.0 / HD, EPS, op0=MUL, op1=ADD)
                y = stpool.tile([128, 2 * HPC], F32, tag="y")
                nc.vector.tensor_scalar(y, m, -0.5, 1.5, op0=MUL, op1=ADD)
                t2 = stpool.tile([128, 2 * HPC], F32, tag="t2")
                for _ in range(2):
                    nc.vector.tensor_mul(t2, y, y)
                    nc.vector.tensor_mul(t2, t2, m)
                    nc.vector.tensor_scalar(t2, t2, -0.5, 1.5, op0=MUL, op1=ADD)
                    nc.vector.tensor_mul(y, y, t2)

                # apply rstd with one stride-0-broadcast multiply (f32 -> bf16)
                qkb = qkpool.tile([128, QK], BF16, tag="qkb")
                nc.vector.tensor_tensor(
                    qkb.rearrange("p (g d) -> p g d", d=HD),
                    qk_sb.rearrange("p (g d) -> p g d", d=HD),
                    bcast_inner(y, HD), op=MUL,
                )

                qkb_prev = (qkb, t)

            qk_transposes(*qkb_prev)

        # ---------------- phase B: attention (+ projection, shared pools) ----
        with ExitStack() as bctx:
            spool = bctx.enter_context(tc.tile_pool(name="ps", bufs=2, space="PSUM"))
            opool = bctx.enter_context(tc.tile_pool(name="po", bufs=1, space="PSUM"))
            ppool = bctx.enter_context(tc.tile_pool(name="pp", bufs=2, space="PSUM"))
            ptpool = bctx.enter_context(tc.tile_pool(name="pt", bufs=3))
            rpool = bctx.enter_context(tc.tile_pool(name="rec", bufs=2))
            outpool = bctx.enter_context(tc.tile_pool(name="outsb", bufs=4))

            for qh in range(2):            # 1024-token halves of the query axis
                for h in range(HPC):
                    cb = h // 2            # column block in qT / pair block in aT
                    # even head: o_un^T rows 0:64, denom rows 64:128; odd: swapped
                    osl = slice(0, 64) if h % 2 == 0 else slice(64, 128)
                    dsl = slice(64, 128) if h % 2 == 0 else slice(0, 64)
                    po = opool.tile([128, 2, 512], F32, tag="po")
                    prev = None
                    for kt in range(NT):
                        ps = spool.tile([128, 1024], F32, tag="ps")
                        for i in range(2):
                            qsl = slice(qh * 1024 + i * 512, qh * 1024 + (i + 1) * 512)
                            nc.tensor.matmul(
                                ps[:, i * 512:(i + 1) * 512],
                                zkT[:, h, kt, :],
                                qT[:, cb, qsl],
                                start=True, stop=True,
                            )
                        pt = ptpool.tile([128, 1024], BF16, tag="pt")
                        nc.scalar.activation(pt, ps, AF.Exp, scale=0.125)
                        if prev is not None:
                            ppt, pkt = prev
                            for i in range(2):
                                nc.tensor.matmul(
                                    po[:, i, :], vaug[:, pkt, h, :],
                                    ppt[:, i * 512:(i + 1) * 512],
                                    start=(pkt == 0), stop=False,
                                )
                        prev = (pt, kt)
                    ppt, pkt = prev
                    for i in range(2):
                        nc.tensor.matmul(
                            po[:, i, :], vaug[:, pkt, h, :],
                            ppt[:, i * 512:(i + 1) * 512],
                            start=False, stop=True,
                        )
                    # normalize: aT rows osl = po rows osl * recip(po rows dsl)
                    rec = rpool.tile([128, 2, 512], F32, tag="rec")
                    nc.vector.reciprocal_approx_fast(rec, po)
                    nc.vector.tensor_mul(
                        aT[osl, cb, qh * 1024:(qh + 1) * 1024]
                          .rearrange("p (i q) -> p i q", i=2),
                        po[osl, :, :],
                        rec[dsl, :, :],
                    )

                # ---- projection for this query half (overlaps next half) ----
                for t in range(qh * NT // 2, (qh + 1) * NT // 2):
                    ts = slice(t * 128, (t + 1) * 128)
                    for jg in range(2):
                        pp = ppool.tile([128, 512], F32, tag="pp")
                        for hb in range(2):
                            nc.tensor.matmul(
                                pp, aT[:, hb, ts],
                                wproj_sb[:, hb, jg * 512:(jg + 1) * 512],
                                start=(hb == 0), stop=(hb == 1),
                            )
                        outsb = outpool.tile([128, 512], F32, tag="outsb")
                        nc.vector.tensor_copy(outsb, pp)
                        nc.sync.dma_start(
                            out=out_ext[ts, jg * 512:(jg + 1) * 512], in_=outsb
                        )

        if dbg:
            with tc.tile_pool(name="dbgp", bufs=1) as dp:
                for nm, dst, src in (("q", dbg_q, qT), ("a", dbg_a, aT)):
                    f = dp.tile([128, 2, N], F32, tag=f"f{nm}", name=f"f{nm}")
                    nc.vector.tensor_copy(f, src)
                    nc.sync.dma_start(out=dst[:, :, :], in_=f)
                fk = dp.tile([128, HPC, NT, 128], F32, tag="fk", name="fk")
                nc.vector.tensor_copy(fk, zkT)
                nc.sync.dma_start(out=dbg_k[:, :, :, :], in_=fk)
                fv = dp.tile([128, NT, HPC, 128], F32, tag="fv", name="fv")
                nc.vector.tensor_copy(fv, vaug)
                nc.sync.dma_start(out=dbg_v[:, :, :, :], in_=fv)

    nc.finalize()
    return nc


def make_in_maps(x, qkv_w, qkv_b, q_norm_w, k_norm_w, proj_w, proj_b):
    """Shard the full inputs into the 8 per-core input maps."""
    bf = ml_dtypes.bfloat16
    in_maps = []
    for c in range(8):
        b, g = c // 4, c % 4
        ch = np.arange(4 * g * HD, 4 * (g + 1) * HD)          # this core's head channels
        wqkv_c = np.concatenate(
            [qkv_w[:, ch], qkv_w[:, C + ch], qkv_w[:, 2 * C + ch]], axis=1
        )
        bqkv_c = np.concatenate([qkv_b[ch], qkv_b[C + ch], qkv_b[2 * C + ch]])
        normw = np.concatenate([np.tile(q_norm_w, HPC), np.tile(k_norm_w, HPC)])
        # wproj rows for this core as [128 rows of head-pair, pair, C]
        wproj_c = proj_w[ch, :].reshape(2, V // 2, C).transpose(1, 0, 2)
        in_maps.append({
            "x": np.ascontiguousarray(x[b]).astype(bf),
            "wqkv": np.ascontiguousarray(wqkv_c).astype(bf),
            "bqkv": np.ascontiguousarray(bqkv_c, np.float32),
            "normw": np.ascontiguousarray(normw, np.float32),
            "wproj": np.ascontiguousarray(wproj_c).astype(bf),
        })
    return in_maps


_NC_CACHE = []


def kernel(x, qkv_w, qkv_b, q_norm_w, k_norm_w, proj_w, proj_b,
           _run_kwargs=None, _res_box=None):
    x = np.asarray(x); qkv_w = np.asarray(qkv_w); qkv_b = np.asarray(qkv_b)
    q_norm_w = np.asarray(q_norm_w); k_norm_w = np.asarray(k_norm_w)
    proj_w = np.asarray(proj_w); proj_b = np.asarray(proj_b)

    if not _NC_CACHE:
        _NC_CACHE.append(build_nc())
    nc = _NC_CACHE[0]
    in_maps = make_in_maps(x, qkv_w, qkv_b, q_norm_w, k_norm_w, proj_w, proj_b)
    res = run_bass_kernel_spmd(nc, in_maps, core_ids=list(range(8)),
                               **(_run_kwargs or {}))
    if _res_box is not None:
        _res_box["res"] = res
    out = np.zeros((B, N, C), np.float32)
    for c in range(8):
        out[c // 4] += res.results[c]["out"]
    out += proj_b[None, None, :].astype(np.float32)
    return out


if __name__ == "__main__":
    rng = np.random.default_rng(0)
    x = rng.standard_normal((B, N, C)).astype(np.float32)
    qkv_w = (rng.standard_normal((C, 3 * C)) / np.sqrt(C)).astype(np.float32)
    qkv_b = np.zeros((3 * C,), np.float32)
    qn = np.ones((HD,), np.float32)
    kn = np.ones((HD,), np.float32)
    proj_w = (rng.standard_normal((C, C)) / np.sqrt(C)).astype(np.float32)
    proj_b = np.zeros((C,), np.float32)
    out = kernel(x, qkv_w, qkv_b, qn, kn, proj_w, proj_b)
    print("out", out.shape, out.dtype, float(np.abs(out).mean()))


# revision 36
# speedup vs baseline: 1.8481x; 1.0234x over previous
"""Fused multi-head attention block on 8 TRN2 NeuronCores.

reference: qkv = x@Wqkv+b; q,k rmsnorm'd per head; softmax(q k^T/sqrt(hd)) v; proj.
Shapes: x [2,2048,1024], H=16 heads, hd=64.

Distribution (no collectives): 8 cores = 2 batches x 4 head-groups (4 heads each).
Core c: batch b=c//4, heads 4g..4g+3 (g=c%4). Each core computes the partial
projection output (proj_w row-sharded over its heads) for its batch; the host
sums the 4 partials per batch and adds proj_b.

Per-core pipeline (bf16 matmul operands, f32 PSUM accumulation):
  A) x (bf16, host-converted) -> PE-transpose (4 packed per PSUM bank) -> xT;
     qkv GEMM; rmsnorm(q,k) with rstd on DVE only (seed (3-m)/2 + Newton);
     one stride-0-broadcast multiply applies rstd; PE-transpose normed qk ->
     qT,kT bf16 (norm weights folded into the evict); v (+bias) -> vaug.
  B) per (head, qtok-half 1024): software-pipelined over ktiles: scores
     S^T(kt) = kT^T qT (2 MMs), exp(kt) on ACT (scale=1/8, no max subtract,
     scores in [-6,6]), AV(kt-1) accumulates with the REUSED stationary
     vaug = [v|ones] (even heads) / [ones|v] (odd heads): out rows give the
     unnormalized out^T on one 64-partition half and the broadcast softmax
     denominator on the other. Epilogue: aT = out^T * approx-recip(denom).
     Head parity places outputs so aT packs head pairs on full 128 partitions.
  C) partial projection straight from aT (2 MMs, K=128), DMA out.
"""

from contextlib import ExitStack

import ml_dtypes
import numpy as np

import concourse.bass as bass
import concourse.mybir as mybir
import concourse.tile as tile
from concourse import bacc
from concourse.bass_utils import run_bass_kernel_spmd
from concourse.masks import make_identity

B, N, C = 2, 2048, 1024
H, HD = 16, 64
HPC = 4                 # heads per core
NT = N // 128           # 16 token tiles
KT8 = C // 128          # 8 contraction tiles for the qkv GEMM
QK = 2 * HPC * HD       # 512 qk channels per core
V = HPC * HD            # 256 v channels per core
EPS = 1e-6
F32 = mybir.dt.float32
BF16 = mybir.dt.bfloat16
AF = mybir.ActivationFunctionType
MUL = mybir.AluOpType.mult
ADD = mybir.AluOpType.add


def bcast_inner(ap, n):
    """Append a stride-0 inner dim of size n to a 2D AP."""
    return bass.AP(tensor=ap.tensor, offset=ap.offset,
                   ap=[list(ap.ap[0]), list(ap.ap[1]), [0, n]])


def build_nc(dbg=False):
    nc = bacc.Bacc("TRN2", target_bir_lowering=False, debug=False)

    x_ext = nc.declare_dram_parameter("x", [C, N], BF16, isOutput=False)
    wqkv_ext = nc.declare_dram_parameter("wqkv", [C, QK + V], BF16, isOutput=False)
    bqkv_ext = nc.declare_dram_parameter("bqkv", [QK + V], F32, isOutput=False)
    normw_ext = nc.declare_dram_parameter("normw", [QK], F32, isOutput=False)
    wproj_ext = nc.declare_dram_parameter("wproj", [V // 2, 2, C], BF16, isOutput=False)
    out_ext = nc.declare_dram_parameter("out", [N, C], F32, isOutput=True)
    if dbg:
        dbg_q = nc.declare_dram_parameter("dbg_q", [128, 2, N], F32, isOutput=True)
        dbg_k = nc.declare_dram_parameter("dbg_k", [128, HPC, NT, 128], F32, isOutput=True)
        dbg_v = nc.declare_dram_parameter("dbg_v", [128, NT, HPC, 128], F32, isOutput=True)
        dbg_a = nc.declare_dram_parameter("dbg_a", [128, 2, N], F32, isOutput=True)

    with tile.TileContext(nc) as tc, ExitStack() as ctx:
        singles = ctx.enter_context(tc.tile_pool(name="singles", bufs=1))

        ident = singles.tile([128, 128], BF16, tag="ident")
        make_identity(nc, ident)

        # host-pre-transposed x, fully resident: [C-partition, kt-block, token]
        xt_all = singles.tile([128, KT8, N], BF16, tag="xt_all")
        nc.sync.dma_start(out=xt_all,
                          in_=x_ext[:, :].rearrange("(kt p) j -> p kt j", p=128))

        wqkv_sb = singles.tile([128, KT8, QK + V], BF16, tag="wqkv")
        for kt in range(KT8):
            nc.sync.dma_start(
                out=wqkv_sb[:, kt, :], in_=wqkv_ext[kt * 128:(kt + 1) * 128, :]
            )
        wproj_sb = singles.tile([128, 2, C], BF16, tag="wproj")
        nc.sync.dma_start(out=wproj_sb, in_=wproj_ext[:, :, :])
        bias_sb = singles.tile([128, QK + V], F32, tag="bias")
        nc.sync.dma_start(out=bias_sb, in_=bqkv_ext[:].partition_broadcast(128))
        # normw_sb[p, cb] = normw[cb*128 + p]
        normw_sb = singles.tile([128, 4], F32, tag="normw")
        nc.sync.dma_start(out=normw_sb, in_=normw_ext[:].rearrange("(b p) -> p b", p=128))

        # persistent activations
        qT = singles.tile([128, 2, N], BF16, tag="qT")     # channel-major q
        # k^T stored per head, zero-padded to K=128 on the partition axis:
        # head h occupies rows (h%2)*64..+64 of [:, h, kt, :]; other rows stay
        # zero so the scores matmul runs with a full-K stationary (K=64
        # matmuls execute at half rate and keep the PE clock cold).
        zkT = singles.tile([128, HPC, NT, 128], BF16, tag="zkT")
        nc.vector.memset(zkT, 0.0)
        # AV stationary blocks: even head s: [v_s | ones]; odd: [ones | v_s]
        vaug = singles.tile([128, NT, HPC, 128], BF16, tag="vaug")
        nc.vector.memset(vaug, 1.0)
        # normalized attention output, transposed; block hb packs heads 2hb,2hb+1
        aT = singles.tile([128, 2, N], BF16, tag="aT")

        # ---------------- phase A: qkv + rmsnorm + transposes ----------------
        with ExitStack() as actx:
            qkpool = actx.enter_context(tc.tile_pool(name="qksb", bufs=4))
            stpool = actx.enter_context(tc.tile_pool(name="stats", bufs=8))
            ptr = actx.enter_context(tc.tile_pool(name="ptr", bufs=3, space="PSUM"))
            pqk = actx.enter_context(tc.tile_pool(name="pqk", bufs=2, space="PSUM"))
            pv = actx.enter_context(tc.tile_pool(name="pv", bufs=2, space="PSUM"))

            def qk_transposes(qkb, t):
                """Transpose normed qk of tile t into qT/zkT (norm w folded)."""
                ts = slice(t * 128, (t + 1) * 128)
                for half in range(2):
                    p_tr = ptr.tile([128, 4, 128], BF16, tag="ptr")
                    for j in range(2):
                        cb = half * 2 + j
                        nc.tensor.matmul(
                            p_tr[:, j, :], qkb[:, cb * 128:(cb + 1) * 128], ident,
                            is_transpose=True, start=(j == 0), stop=(j == 1),
                        )
                    for j in range(2):
                        cb = half * 2 + j
                        if cb < 2:
                            nc.vector.tensor_scalar_mul(qT[:, cb, ts], p_tr[:, j, :],
                                                        normw_sb[:, cb:cb + 1])
                        else:
                            kb = cb - 2    # head pair block
                            for par in range(2):
                                rows = slice(par * 64, par * 64 + 64)
                                if par == 0:
                                    nc.vector.tensor_scalar_mul(
                                        zkT[rows, 2 * kb, t, :], p_tr[rows, j, :],
                                        normw_sb[rows, cb:cb + 1])
                                else:
                                    nc.scalar.activation(
                                        zkT[rows, 2 * kb + par, t, :],
                                        p_tr[rows, j, :],
                                        AF.Copy, scale=normw_sb[rows, cb:cb + 1],
                                    )

            qkb_prev = None
            for t in range(NT):
                ts = slice(t * 128, (t + 1) * 128)

                # qkv GEMM for this token tile (x^T comes pre-transposed)
                p_qk = pqk.tile([128, QK], F32, tag="pqk")
                p_v = pv.tile([128, V], F32, tag="pv")
                for kt in range(KT8):
                    nc.tensor.matmul(
                        p_qk, xt_all[:, kt, ts], wqkv_sb[:, kt, 0:QK],
                        start=(kt == 0), stop=(kt == KT8 - 1),
                    )
                for kt in range(KT8):
                    nc.tensor.matmul(
                        p_v, xt_all[:, kt, ts], wqkv_sb[:, kt, QK:QK + V],
                        start=(kt == 0), stop=(kt == KT8 - 1),
                    )

                # previous tile's qk transposes go here so the PE never waits
                # on this tile's rmsnorm DVE chain (in-order queue)
                if qkb_prev is not None:
                    qk_transposes(*qkb_prev)

                # evictions with bias add; v goes to its parity-dependent half
                qk_sb = qkpool.tile([128, QK], F32, tag="qksb")
                nc.vector.tensor_add(qk_sb, p_qk, bias_sb[:, 0:QK])
                pv3 = p_v.rearrange("p (h d) -> p h d", d=HD)
                bv3 = bias_sb[:, QK:QK + V].rearrange("p (h d) -> p h d", d=HD)
                nc.vector.tensor_add(vaug[:, t, 0::2, 0:HD], pv3[:, 0::2, :],
                                     bv3[:, 0::2, :])
                nc.vector.tensor_add(vaug[:, t, 1::2, HD:128], pv3[:, 1::2, :],
                                     bv3[:, 1::2, :])

                # rmsnorm: rstd = 1/sqrt(m), m = ssq/64 + eps:
                # seed (3-m)/2 then 2 Newton steps y *= 1.5 - 0.5*m*y^2
                sq = qkpool.tile([128, QK], F32, tag="sq")
                nc.vector.tensor_mul(sq, qk_sb, qk_sb)
                m = stpool.tile([128, 2 * HPC], F32, tag="m")
                nc.vector.tensor_reduce(
                    m, sq.rearrange("p (g d) -> p g d", d=HD),
                    axis=mybir.AxisListType.X, op=mybir.AluOpType.add,
                )
                nc.vector.tensor_scalar(m, m, 1.0 / HD, EPS, op0=MUL, op1=ADD)
                y = stpool.tile([128, 2 * HPC], F32, tag="y")
                nc.vector.tensor_scalar(y, m, -0.5, 1.5, op0=MUL, op1=ADD)
                t2 = stpool.tile([128, 2 * HPC], F32, tag="t2")
                for _ in range(2):
                    nc.vector.tensor_mul(t2, y, y)
                    nc.vector.tensor_mul(t2, t2, m)
                    nc.vector.tensor_scalar(t2, t2, -0.5, 1.5, op0=MUL, op1=ADD)
                    nc.vector.tensor_mul(y, y, t2)

                # apply rstd with one stride-0-broadcast multiply (f32 -> bf16)
                qkb = qkpool.tile([128, QK], BF16, tag="qkb")
                nc.vector.tensor_tensor(
                    qkb.rearrange("p (g d) -> p g d", d=HD),
                    qk_sb.rearrange("p (g d) -> p g d", d=HD),
                    bcast_inner(y, HD), op=MUL,
                )

                qkb_prev = (qkb, t)

            qk_transposes(*qkb_prev)

        # ---------------- phase B: attention (+ projection, shared pools) ----
        with ExitStack() as bctx:
            spool = bctx.enter_context(tc.tile_pool(name="ps", bufs=2, space="PSUM"))
            opool = bctx.enter_context(tc.tile_pool(name="po", bufs=1, space="PSUM"))
            ppool = bctx.enter_context(tc.tile_pool(name="pp", bufs=2, space="PSUM"))
            ptpool = bctx.enter_context(tc.tile_pool(name="pt", bufs=3))
            rpool = bctx.enter_context(tc.tile_pool(name="rec", bufs=2))
            outpool = bctx.enter_context(tc.tile_pool(name="outsb", bufs=4))

            for qh in range(2):            # 1024-token halves of the query axis
                for h in range(HPC):
                    cb = h // 2            # column block in qT / pair block in aT
                    # even head: o_un^T rows 0:64, denom rows 64:128; odd: swapped
                    osl = slice(0, 64) if h % 2 == 0 else slice(64, 128)
                    dsl = slice(64, 128) if h % 2 == 0 else slice(0, 64)
                    po = opool.tile([128, 2, 512], F32, tag="po")
                    prev = None
                    for kt in range(NT):
                        ps = spool.tile([128, 1024], F32, tag="ps")
                        for i in range(2):
                            qsl = slice(qh * 1024 + i * 512, qh * 1024 + (i + 1) * 512)
                            nc.tensor.matmul(
                                ps[:, i * 512:(i + 1) * 512],
                                zkT[:, h, kt, :],
                                qT[:, cb, qsl],
                                start=True, stop=True,
                            )
                        pt = ptpool.tile([128, 1024], BF16, tag="pt")
                        nc.scalar.activation(pt, ps, AF.Exp, scale=0.125)
                        if prev is not None:
                            ppt, pkt = prev
                            for i in range(2):
                                nc.tensor.matmul(
                                    po[:, i, :], vaug[:, pkt, h, :],
                                    ppt[:, i * 512:(i + 1) * 512],
                                    start=(pkt == 0), stop=False,
                                )
                        prev = (pt, kt)
                    ppt, pkt = prev
                    for i in range(2):
                        nc.tensor.matmul(
                            po[:, i, :], vaug[:, pkt, h, :],
                            ppt[:, i * 512:(i + 1) * 512],
                            start=False, stop=True,
                        )
                    # normalize: aT rows osl = po rows osl * recip(po rows dsl)
                    rec = rpool.tile([128, 2, 512], F32, tag="rec")
                    nc.vector.reciprocal_approx_fast(rec, po)
                    nc.vector.tensor_mul(
                        aT[osl, cb, qh * 1024:(qh + 1) * 1024]
                          .rearrange("p (i q) -> p i q", i=2),
                        po[osl, :, :],
                        rec[dsl, :, :],
                    )

                # ---- projection for this query half (overlaps next half) ----
                for t in range(qh * NT // 2, (qh + 1) * NT // 2):
                    ts = slice(t * 128, (t + 1) * 128)
                    for jg in range(2):
                        pp = ppool.tile([128, 512], F32, tag="pp")
                        for hb in range(2):
                            nc.tensor.matmul(
                                pp, aT[:, hb, ts],
                                wproj_sb[:, hb, jg * 512:(jg + 1) * 512],
                                start=(hb == 0), stop=(hb == 1),
                            )
                        outsb = outpool.tile([128, 512], F32, tag="outsb")
                        nc.vector.tensor_copy(outsb, pp)
                        nc.sync.dma_start(
                            out=out_ext[ts, jg * 512:(jg + 1) * 512], in_=outsb
                        )

        if dbg:
            with tc.tile_pool(name="dbgp", bufs=1) as dp:
                for nm, dst, src in (("q", dbg_q, qT), ("a", dbg_a, aT)):
                    f = dp.tile([128, 2, N], F32, tag=f"f{nm}", name=f"f{nm}")
                    nc.vector.tensor_copy(f, src)
                    nc.sync.dma_start(out=dst[:, :, :], in_=f)
                fk = dp.tile([128, HPC, NT, 128], F32, tag="fk", name="fk")
                nc.vector.tensor_copy(fk, zkT)
                nc.sync.dma_start(out=dbg_k[:, :, :, :], in_=fk)
                fv = dp.tile([128, NT, HPC, 128], F32, tag="fv", name="fv")
                nc.vector.tensor_copy(fv, vaug)
                nc.sync.dma_start(out=dbg_v[:, :, :, :], in_=fv)

    nc.finalize()
    return nc


def make_in_maps(x, qkv_w, qkv_b, q_norm_w, k_norm_w, proj_w, proj_b):
    """Shard the full inputs into the 8 per-core input maps."""
    bf = ml_dtypes.bfloat16
    in_maps = []
    for c in range(8):
        b, g = c // 4, c % 4
        ch = np.arange(4 * g * HD, 4 * (g + 1) * HD)          # this core's head channels
        wqkv_c = np.concatenate(
            [qkv_w[:, ch], qkv_w[:, C + ch], qkv_w[:, 2 * C + ch]], axis=1
        )
        bqkv_c = np.concatenate([qkv_b[ch], qkv_b[C + ch], qkv_b[2 * C + ch]])
        normw = np.concatenate([np.tile(q_norm_w, HPC), np.tile(k_norm_w, HPC)])
        # wproj rows for this core as [128 rows of head-pair, pair, C]
        wproj_c = proj_w[ch, :].reshape(2, V // 2, C).transpose(1, 0, 2)
        in_maps.append({
            "x": np.ascontiguousarray(x[b].T).astype(bf),
            "wqkv": np.ascontiguousarray(wqkv_c).astype(bf),
            "bqkv": np.ascontiguousarray(bqkv_c, np.float32),
            "normw": np.ascontiguousarray(normw, np.float32),
            "wproj": np.ascontiguousarray(wproj_c).astype(bf),
        })
    return in_maps


_NC_CACHE = []


def kernel(x, qkv_w, qkv_b, q_norm_w, k_norm_w, proj_w, proj_b,
           _run_kwargs=None, _res_box=None):
    x = np.asarray(x); qkv_w = np.asarray(qkv_w); qkv_b = np.asarray(qkv_b)
    q_norm_w = np.asarray(q_norm_w); k_norm_w = np.asarray(k_norm_w)
    proj_w = np.asarray(proj_w); proj_b = np.asarray(proj_b)

    if not _NC_CACHE:
        _NC_CACHE.append(build_nc())
    nc = _NC_CACHE[0]
    in_maps = make_in_maps(x, qkv_w, qkv_b, q_norm_w, k_norm_w, proj_w, proj_b)
    res = run_bass_kernel_spmd(nc, in_maps, core_ids=list(range(8)),
                               **(_run_kwargs or {}))
    if _res_box is not None:
        _res_box["res"] = res
    out = np.zeros((B, N, C), np.float32)
    for c in range(8):
        out[c // 4] += res.results[c]["out"]
    out += proj_b[None, None, :].astype(np.float32)
    return out


if __name__ == "__main__":
    rng = np.random.default_rng(0)
    x = rng.standard_normal((B, N, C)).astype(np.float32)
    qkv_w = (rng.standard_normal((C, 3 * C)) / np.sqrt(C)).astype(np.float32)
    qkv_b = np.zeros((3 * C,), np.float32)
    qn = np.ones((HD,), np.float32)
    kn = np.ones((HD,), np.float32)
    proj_w = (rng.standard_normal((C, C)) / np.sqrt(C)).astype(np.float32)
    proj_b = np.zeros((C,), np.float32)
    out = kernel(x, qkv_w, qkv_b, qn, kn, proj_w, proj_b)
    print("out", out.shape, out.dtype, float(np.abs(out).mean()))


# revision 39
# speedup vs baseline: 1.9173x; 1.0374x over previous
"""Fused multi-head attention block on 8 TRN2 NeuronCores.

reference: qkv = x@Wqkv+b; q,k rmsnorm'd per head; softmax(q k^T/sqrt(hd)) v; proj.
Shapes: x [2,2048,1024], H=16 heads, hd=64.

Distribution (no collectives): 8 cores = 2 batches x 4 head-groups (4 heads each).
Core c: batch b=c//4, heads 4g..4g+3 (g=c%4). Each core computes the partial
projection output (proj_w row-sharded over its heads) for its batch; the host
sums the 4 partials per batch and adds proj_b.

Per-core pipeline (bf16 matmul operands, f32 PSUM accumulation):
  A) x (bf16, host-converted) -> PE-transpose (4 packed per PSUM bank) -> xT;
     qkv GEMM; rmsnorm(q,k) with rstd on DVE only (seed (3-m)/2 + Newton);
     one stride-0-broadcast multiply applies rstd; PE-transpose normed qk ->
     qT,kT bf16 (norm weights folded into the evict); v (+bias) -> vaug.
  B) per (head, qtok-half 1024): software-pipelined over ktiles: scores
     S^T(kt) = kT^T qT (2 MMs), exp(kt) on ACT (scale=1/8, no max subtract,
     scores in [-6,6]), AV(kt-1) accumulates with the REUSED stationary
     vaug = [v|ones] (even heads) / [ones|v] (odd heads): out rows give the
     unnormalized out^T on one 64-partition half and the broadcast softmax
     denominator on the other. Epilogue: aT = out^T * approx-recip(denom).
     Head parity places outputs so aT packs head pairs on full 128 partitions.
  C) partial projection straight from aT (2 MMs, K=128), DMA out.
"""

from contextlib import ExitStack

import ml_dtypes
import numpy as np

import concourse.bass as bass
import concourse.mybir as mybir
import concourse.tile as tile
from concourse import bacc
from concourse.bass_utils import run_bass_kernel_spmd
from concourse.masks import make_identity

B, N, C = 2, 2048, 1024
H, HD = 16, 64
HPC = 4                 # heads per core
NT = N // 128           # 16 token tiles
KT8 = C // 128          # 8 contraction tiles for the qkv GEMM
QK = 2 * HPC * HD       # 512 qk channels per core
V = HPC * HD            # 256 v channels per core
EPS = 1e-6
F32 = mybir.dt.float32
BF16 = mybir.dt.bfloat16
AF = mybir.ActivationFunctionType
MUL = mybir.AluOpType.mult
ADD = mybir.AluOpType.add


def bcast_inner(ap, n):
    """Append a stride-0 inner dim of size n to a 2D AP."""
    return bass.AP(tensor=ap.tensor, offset=ap.offset,
                   ap=[list(ap.ap[0]), list(ap.ap[1]), [0, n]])


def build_nc(dbg=False):
    nc = bacc.Bacc("TRN2", target_bir_lowering=False, debug=False)

    x_ext = nc.declare_dram_parameter("x", [C, N], BF16, isOutput=False)
    wqkv_ext = nc.declare_dram_parameter("wqkv", [C, QK + V], BF16, isOutput=False)
    bqkv_ext = nc.declare_dram_parameter("bqkv", [QK + V], F32, isOutput=False)
    normw_ext = nc.declare_dram_parameter("normw", [QK], F32, isOutput=False)
    wproj_ext = nc.declare_dram_parameter("wproj", [V // 2, 2, C], BF16, isOutput=False)
    out_ext = nc.declare_dram_parameter("out", [N, C], F32, isOutput=True)
    if dbg:
        dbg_q = nc.declare_dram_parameter("dbg_q", [128, 2, N], F32, isOutput=True)
        dbg_k = nc.declare_dram_parameter("dbg_k", [128, HPC, NT, 128], F32, isOutput=True)
        dbg_v = nc.declare_dram_parameter("dbg_v", [128, NT, HPC, 128], F32, isOutput=True)
        dbg_a = nc.declare_dram_parameter("dbg_a", [128, 2, N], F32, isOutput=True)

    with tile.TileContext(nc) as tc, ExitStack() as ctx:
        singles = ctx.enter_context(tc.tile_pool(name="singles", bufs=1))

        ident = singles.tile([128, 128], BF16, tag="ident")
        make_identity(nc, ident)

        # host-pre-transposed x, fully resident: [C-partition, kt-block, token]
        xt_all = singles.tile([128, KT8, N], BF16, tag="xt_all")
        for js in range(4):
            jsl = slice(js * (N // 4), (js + 1) * (N // 4))
            nc.sync.dma_start(
                out=xt_all[:, :, jsl],
                in_=x_ext[:, jsl].rearrange("(kt p) j -> p kt j", p=128))

        wqkv_sb = singles.tile([128, KT8, QK + V], BF16, tag="wqkv")
        for kt in range(KT8):
            nc.sync.dma_start(
                out=wqkv_sb[:, kt, :], in_=wqkv_ext[kt * 128:(kt + 1) * 128, :]
            )
        wproj_sb = singles.tile([128, 2, C], BF16, tag="wproj")
        nc.sync.dma_start(out=wproj_sb, in_=wproj_ext[:, :, :])
        bias_sb = singles.tile([128, QK + V], F32, tag="bias")
        nc.sync.dma_start(out=bias_sb, in_=bqkv_ext[:].partition_broadcast(128))
        # normw_sb[p, cb] = normw[cb*128 + p]
        normw_sb = singles.tile([128, 4], F32, tag="normw")
        nc.sync.dma_start(out=normw_sb, in_=normw_ext[:].rearrange("(b p) -> p b", p=128))

        # persistent activations
        qT = singles.tile([128, 2, N], BF16, tag="qT")     # channel-major q
        # k^T stored per head, zero-padded to K=128 on the partition axis:
        # head h occupies rows (h%2)*64..+64 of [:, h, kt, :]; other rows stay
        # zero so the scores matmul runs with a full-K stationary (K=64
        # matmuls execute at half rate and keep the PE clock cold).
        zkT = singles.tile([128, HPC, NT, 128], BF16, tag="zkT")
        nc.gpsimd.memset(zkT, 0.0)
        # AV stationary blocks: even head s: [v_s | ones]; odd: [ones | v_s]
        vaug = singles.tile([128, NT, HPC, 128], BF16, tag="vaug")
        nc.gpsimd.memset(vaug, 1.0)
        # normalized attention output, transposed; block hb packs heads 2hb,2hb+1
        aT = singles.tile([128, 2, N], BF16, tag="aT")

        # ---------------- phase A: qkv + rmsnorm + transposes ----------------
        with ExitStack() as actx:
            qkpool = actx.enter_context(tc.tile_pool(name="qksb", bufs=5))
            stpool = actx.enter_context(tc.tile_pool(name="stats", bufs=8))
            ptr = actx.enter_context(tc.tile_pool(name="ptr", bufs=3, space="PSUM"))
            pqk = actx.enter_context(tc.tile_pool(name="pqk", bufs=2, space="PSUM"))
            pv = actx.enter_context(tc.tile_pool(name="pv", bufs=2, space="PSUM"))

            def qk_transposes(qkb, t):
                """Transpose normed qk of tile t into qT/zkT (norm w folded)."""
                ts = slice(t * 128, (t + 1) * 128)
                for half in range(2):
                    p_tr = ptr.tile([128, 4, 128], BF16, tag="ptr")
                    for j in range(2):
                        cb = half * 2 + j
                        nc.tensor.matmul(
                            p_tr[:, j, :], qkb[:, cb * 128:(cb + 1) * 128], ident,
                            is_transpose=True, start=(j == 0), stop=(j == 1),
                        )
                    for j in range(2):
                        cb = half * 2 + j
                        if cb < 2:
                            nc.scalar.activation(qT[:, cb, ts], p_tr[:, j, :],
                                                 AF.Copy,
                                                 scale=normw_sb[:, cb:cb + 1])
                        else:
                            kb = cb - 2    # head pair block
                            for par in range(2):
                                rows = slice(par * 64, par * 64 + 64)
                                eng = nc.vector if par == 0 else nc.scalar
                                if par == 0:
                                    nc.vector.tensor_scalar_mul(
                                        zkT[rows, 2 * kb, t, :], p_tr[rows, j, :],
                                        normw_sb[rows, cb:cb + 1])
                                else:
                                    nc.scalar.activation(
                                        zkT[rows, 2 * kb + par, t, :],
                                        p_tr[rows, j, :],
                                        AF.Copy, scale=normw_sb[rows, cb:cb + 1],
                                    )

            qkb_prev = []
            pend = None
            for t in range(NT):
                ts = slice(t * 128, (t + 1) * 128)

                # qkv GEMM for this token tile (x^T comes pre-transposed)
                p_qk = pqk.tile([128, QK], F32, tag="pqk")
                p_v = pv.tile([128, V], F32, tag="pv")
                for kt in range(KT8):
                    nc.tensor.matmul(
                        p_qk, xt_all[:, kt, ts], wqkv_sb[:, kt, 0:QK],
                        start=(kt == 0), stop=(kt == KT8 - 1),
                    )
                for kt in range(KT8):
                    nc.tensor.matmul(
                        p_v, xt_all[:, kt, ts], wqkv_sb[:, kt, QK:QK + V],
                        start=(kt == 0), stop=(kt == KT8 - 1),
                    )

                # earlier tiles' qk transposes go here so the PE never waits
                # on the rmsnorm DVE chain (in-order queue)
                for args in qkb_prev:
                    qk_transposes(*args)
                qkb_prev = []

                # evictions with bias add; v goes to its parity-dependent half
                qk_sb = qkpool.tile([128, QK], F32, tag="qksb")
                nc.vector.tensor_add(qk_sb, p_qk, bias_sb[:, 0:QK])
                pv3 = p_v.rearrange("p (h d) -> p h d", d=HD)
                bv3 = bias_sb[:, QK:QK + V].rearrange("p (h d) -> p h d", d=HD)
                nc.vector.tensor_add(vaug[:, t, 0::2, 0:HD], pv3[:, 0::2, :],
                                     bv3[:, 0::2, :])
                nc.vector.tensor_add(vaug[:, t, 1::2, HD:128], pv3[:, 1::2, :],
                                     bv3[:, 1::2, :])

                if pend is None:
                    pend = (qk_sb, t)
                    continue

                # rmsnorm for the tile pair: rstd = 1/sqrt(m), m = ssq/64+eps;
                # seed (3-m)/2 then 2 Newton steps y *= 1.5 - 0.5*m*y^2,
                # one chain over both tiles' 16 head-groups
                qk_a, t_a = pend
                pend = None
                m = stpool.tile([128, 4 * HPC], F32, tag="m")
                for idx, qsb in ((0, qk_a), (1, qk_sb)):
                    sq = qkpool.tile([128, QK], F32, tag="sq")
                    nc.vector.tensor_mul(sq, qsb, qsb)
                    nc.vector.tensor_reduce(
                        m[:, idx * 8:(idx + 1) * 8],
                        sq.rearrange("p (g d) -> p g d", d=HD),
                        axis=mybir.AxisListType.X, op=mybir.AluOpType.add,
                    )
                nc.vector.tensor_scalar(m, m, 1.0 / HD, EPS, op0=MUL, op1=ADD)
                y = stpool.tile([128, 4 * HPC], F32, tag="y")
                nc.vector.tensor_scalar(y, m, -0.5, 1.5, op0=MUL, op1=ADD)
                t2 = stpool.tile([128, 4 * HPC], F32, tag="t2")
                for _ in range(2):
                    nc.vector.tensor_mul(t2, y, y)
                    nc.vector.tensor_mul(t2, t2, m)
                    nc.vector.tensor_scalar(t2, t2, -0.5, 1.5, op0=MUL, op1=ADD)
                    nc.vector.tensor_mul(y, y, t2)

                for idx, (qsb, tt) in ((0, (qk_a, t_a)), (1, (qk_sb, t))):
                    qkb = qkpool.tile([128, QK], BF16, tag="qkb")
                    nc.vector.tensor_tensor(
                        qkb.rearrange("p (g d) -> p g d", d=HD),
                        qsb.rearrange("p (g d) -> p g d", d=HD),
                        bcast_inner(y[:, idx * 8:(idx + 1) * 8], HD), op=MUL,
                    )
                    qkb_prev.append((qkb, tt))

            for args in qkb_prev:
                qk_transposes(*args)

        # ---------------- phase B: attention (+ projection, shared pools) ----
        with ExitStack() as bctx:
            spool = bctx.enter_context(tc.tile_pool(name="ps", bufs=2, space="PSUM"))
            opool = bctx.enter_context(tc.tile_pool(name="po", bufs=1, space="PSUM"))
            ppool = bctx.enter_context(tc.tile_pool(name="pp", bufs=2, space="PSUM"))
            ptpool = bctx.enter_context(tc.tile_pool(name="pt", bufs=3))
            rpool = bctx.enter_context(tc.tile_pool(name="rec", bufs=2))
            outpool = bctx.enter_context(tc.tile_pool(name="outsb", bufs=4))

            for qh in range(2):            # 1024-token halves of the query axis
                for h in range(HPC):
                    cb = h // 2            # column block in qT / pair block in aT
                    # even head: o_un^T rows 0:64, denom rows 64:128; odd: swapped
                    osl = slice(0, 64) if h % 2 == 0 else slice(64, 128)
                    dsl = slice(64, 128) if h % 2 == 0 else slice(0, 64)
                    po = opool.tile([128, 2, 512], F32, tag="po")
                    prev = None
                    for kt in range(NT):
                        ps = spool.tile([128, 1024], F32, tag="ps")
                        for i in range(2):
                            qsl = slice(qh * 1024 + i * 512, qh * 1024 + (i + 1) * 512)
                            nc.tensor.matmul(
                                ps[:, i * 512:(i + 1) * 512],
                                zkT[:, h, kt, :],
                                qT[:, cb, qsl],
                                start=True, stop=True,
                            )
                        pt = ptpool.tile([128, 1024], BF16, tag="pt")
                        nc.scalar.activation(pt, ps, AF.Exp, scale=0.125)
                        if prev is not None:
                            ppt, pkt = prev
                            for i in range(2):
                                nc.tensor.matmul(
                                    po[:, i, :], vaug[:, pkt, h, :],
                                    ppt[:, i * 512:(i + 1) * 512],
                                    start=(pkt == 0), stop=False,
                                )
                        prev = (pt, kt)
                    ppt, pkt = prev
                    for i in range(2):
                        nc.tensor.matmul(
                            po[:, i, :], vaug[:, pkt, h, :],
                            ppt[:, i * 512:(i + 1) * 512],
                            start=False, stop=True,
                        )
                    # normalize: aT rows osl = po rows osl * recip(po rows dsl)
                    rec = rpool.tile([128, 2, 512], F32, tag="rec")
                    nc.vector.reciprocal_approx_fast(rec, po)
                    nc.vector.tensor_mul(
                        aT[osl, cb, qh * 1024:(qh + 1) * 1024]
                          .rearrange("p (i q) -> p i q", i=2),
                        po[osl, :, :],
                        rec[dsl, :, :],
                    )

                # ---- projection for this query half (overlaps next half) ----
                for t in range(qh * NT // 2, (qh + 1) * NT // 2):
                    ts = slice(t * 128, (t + 1) * 128)
                    for jg in range(2):
                        pp = ppool.tile([128, 512], F32, tag="pp")
                        for hb in range(2):
                            nc.tensor.matmul(
                                pp, aT[:, hb, ts],
                                wproj_sb[:, hb, jg * 512:(jg + 1) * 512],
                                start=(hb == 0), stop=(hb == 1),
                            )
                        outsb = outpool.tile([128, 512], F32, tag="outsb")
                        nc.vector.tensor_copy(outsb, pp)
                        nc.sync.dma_start(
                            out=out_ext[ts, jg * 512:(jg + 1) * 512], in_=outsb
                        )

        if dbg:
            with tc.tile_pool(name="dbgp", bufs=1) as dp:
                for nm, dst, src in (("q", dbg_q, qT), ("a", dbg_a, aT)):
                    f = dp.tile([128, 2, N], F32, tag=f"f{nm}", name=f"f{nm}")
                    nc.vector.tensor_copy(f, src)
                    nc.sync.dma_start(out=dst[:, :, :], in_=f)
                fk = dp.tile([128, HPC, NT, 128], F32, tag="fk", name="fk")
                nc.vector.tensor_copy(fk, zkT)
                nc.sync.dma_start(out=dbg_k[:, :, :, :], in_=fk)
                fv = dp.tile([128, NT, HPC, 128], F32, tag="fv", name="fv")
                nc.vector.tensor_copy(fv, vaug)
                nc.sync.dma_start(out=dbg_v[:, :, :, :], in_=fv)

    nc.finalize()
    return nc


def make_in_maps(x, qkv_w, qkv_b, q_norm_w, k_norm_w, proj_w, proj_b):
    """Shard the full inputs into the 8 per-core input maps."""
    bf = ml_dtypes.bfloat16
    in_maps = []
    for c in range(8):
        b, g = c // 4, c % 4
        ch = np.arange(4 * g * HD, 4 * (g + 1) * HD)          # this core's head channels
        wqkv_c = np.concatenate(
            [qkv_w[:, ch], qkv_w[:, C + ch], qkv_w[:, 2 * C + ch]], axis=1
        )
        bqkv_c = np.concatenate([qkv_b[ch], qkv_b[C + ch], qkv_b[2 * C + ch]])
        normw = np.concatenate([np.tile(q_norm_w, HPC), np.tile(k_norm_w, HPC)])
        # wproj rows for this core as [128 rows of head-pair, pair, C]
        wproj_c = proj_w[ch, :].reshape(2, V // 2, C).transpose(1, 0, 2)
        in_maps.append({
            "x": np.ascontiguousarray(x[b].T).astype(bf),
            "wqkv": np.ascontiguousarray(wqkv_c).astype(bf),
            "bqkv": np.ascontiguousarray(bqkv_c, np.float32),
            "normw": np.ascontiguousarray(normw, np.float32),
            "wproj": np.ascontiguousarray(wproj_c).astype(bf),
        })
    return in_maps


_NC_CACHE = []


def kernel(x, qkv_w, qkv_b, q_norm_w, k_norm_w, proj_w, proj_b,
           _run_kwargs=None, _res_box=None):
    x = np.asarray(x); qkv_w = np.asarray(qkv_w); qkv_b = np.asarray(qkv_b)
    q_norm_w = np.asarray(q_norm_w); k_norm_w = np.asarray(k_norm_w)
    proj_w = np.asarray(proj_w); proj_b = np.asarray(proj_b)

    if not _NC_CACHE:
        _NC_CACHE.append(build_nc())
    nc = _NC_CACHE[0]
    in_maps = make_in_maps(x, qkv_w, qkv_b, q_norm_w, k_norm_w, proj_w, proj_b)
    res = run_bass_kernel_spmd(nc, in_maps, core_ids=list(range(8)),
                               **(_run_kwargs or {}))
    if _res_box is not None:
        _res_box["res"] = res
    out = np.zeros((B, N, C), np.float32)
    for c in range(8):
        out[c // 4] += res.results[c]["out"]
    out += proj_b[None, None, :].astype(np.float32)
    return out


if __name__ == "__main__":
    rng = np.random.default_rng(0)
    x = rng.standard_normal((B, N, C)).astype(np.float32)
    qkv_w = (rng.standard_normal((C, 3 * C)) / np.sqrt(C)).astype(np.float32)
    qkv_b = np.zeros((3 * C,), np.float32)
    qn = np.ones((HD,), np.float32)
    kn = np.ones((HD,), np.float32)
    proj_w = (rng.standard_normal((C, C)) / np.sqrt(C)).astype(np.float32)
    proj_b = np.zeros((C,), np.float32)
    out = kernel(x, qkv_w, qkv_b, qn, kn, proj_w, proj_b)
    print("out", out.shape, out.dtype, float(np.abs(out).mean()))
